# revision 1
# baseline (speedup 1.0000x reference)
"""Trainium2 Bass kernel for CollaborativeAttention.

Math: with S=512 unique positions and F=T=2048 gathered via fpos/tpos (mod 512),
the whole block collapses to the unique-position problem:
    qf = hs @ Wq ; kf = hs @ Wk ; vf = hs @ Wv + bv ; cbf = hs @ Wcb       [512, *]
    per head h:  w[u, s] = counts[s] * exp(scale*(qf[u]*mix[h]) . kf[s]
                                           + scale*cbf[s, h])
    ctx[u, h*64:(h+1)*64] = (w @ vf[:, h*64:(h+1)*64]) / w.sum(axis=1)
    outfull = ctx @ Wd + bd ; resfull = hs + outfull ; LN  -> normedfull   [512, 1024]
    output  = normedfull[fpos % 512]                                       [2048, 1024]
counts[s] = multiplicity of s in (tpos % 512); softmax over the 2048 keys is
exactly the count-weighted softmax over the 512 unique keys.

Distribution: collectives on this stack cost ~1.4 ms each (measured), which
dwarfs the entire collapsed computation (~0.2 ms). So every core runs the full
replicated problem (no collectives); the host takes core 0's output. Matmuls
use float32r (full-rate single-pass fp32) where the moving dim is 512.
"""

import math
import numpy as np

P = 128
S = 512
D = 1024
H = 16
DH = 64
NB = D // P          # 8 contraction chunks
N_CORES = 8
NPAIR = H // 2
SCALE = 1.0 / math.sqrt(D / H)  # 0.125
LN_EPS = 1e-5
NEG_BIG = -30000.0

# use float32r (full-rate single-pass reduced-precision fp32) on the matmul path
F32R = True

_CACHE = {}


def _emit(nc, tc, pools, io, it):
    """Emit one full compute iteration (everything after the constant loads)."""
    import concourse.mybir as mybir

    def wvblk2d(t):
        return t.rearrange("p a b -> p (a b)")[:, :S]

    fp = mybir.dt.float32
    fr = mybir.dt.float32r if F32R else fp
    Alu = mybir.AluOpType
    Act = mybir.ActivationFunctionType

    mqp, wp, wkp, ep, ps, wstream = (pools[k] for k in
                                     ("mqp", "wp", "wkp", "ep", "ps", "wstream"))
    hT = io["hT"]

    # ---- q/k projections (full):  qT = (hs @ Wq)^T, kT likewise ----
    qT = mqp.tile([P, NB, S], fr, tag="qT", name=f"qT{it}", bufs=1)
    kT = mqp.tile([P, NB, S], fr, tag="kT", name=f"kT{it}", bufs=1)
    for wdram, dest in ((io["wq"], qT), (io["wk"], kT)):
        wr = wdram.rearrange("(o p) m -> p o m", p=P)
        for o in range(NB):
            wblk = wstream.tile([P, NB, P], fr, tag="wst", name=f"wblk{it}")
            nc.sync.dma_start(wblk[:], wr[:, :, P * o: P * (o + 1)])
            pt = ps.tile([P, S], fp, tag="ps", name=f"pt{it}")
            for ic in range(NB):
                nc.tensor.matmul(pt[:], lhsT=wblk[:, ic, :],
                                 rhs=hT[:, ic, :],
                                 start=(ic == 0), stop=(ic == NB - 1))
            nc.scalar.copy(dest[:, o, :], pt[:])

    # ---- cb (content bias) for all heads: one psum bank, 4 regions ----
    cb_ps = ps.tile([P, 4, H], fp, tag="ps", name=f"cb_ps{it}")
    for ic in range(NB):
        for st in range(4):
            nc.tensor.matmul(cb_ps[:, st, :],
                             lhsT=hT[:, ic, P * st: P * (st + 1)],
                             rhs=io["wcb_sb"][:, ic, :],
                             start=(ic == 0 and st == 0),
                             stop=(ic == NB - 1 and st == 3),
                             skip_group_check=True)
    # exp bias per key s and head: scale*cb[s, h] + ln(counts[s])
    bias_sb = mqp.tile([P, 4, H], fp, tag="bias", name=f"bias_sb{it}")
    for st in range(4):
        nc.vector.scalar_tensor_tensor(
            out=bias_sb[:, st, :], in0=cb_ps[:, st, :], scalar=SCALE,
            in1=io["lncnt_sb"][:, st:st + 1].to_broadcast([P, H]),
            op0=Alu.mult, op1=Alu.add)

    # ---- v projection (full, streamed by column half) ----
    v_sb = mqp.tile([P, 4, D], fr, tag="v", name=f"v_sb{it}", bufs=1)
    wvr = io["wv"].rearrange("(o p) m -> p o m", p=P)
    for eh in range(2):
        v_ps = [ps.tile([P, S], fp, tag="ps", name=f"v_ps{it}_{st}")
                for st in range(4)]
        for ic in range(NB):
            wvblk = wstream.tile([P, NB, P], fr, tag="wst", name=f"wvblk{it}")
            nc.sync.dma_start(wvblk2d(wvblk)[:], wvr[:, ic, S * eh: S * (eh + 1)])
            for st in range(4):
                nc.tensor.matmul(v_ps[st][:],
                                 lhsT=hT[:, ic, P * st: P * (st + 1)],
                                 rhs=wvblk2d(wvblk)[:],
                                 start=(ic == 0), stop=(ic == NB - 1))
        for st in range(4):
            nc.scalar.copy(v_sb[:, st, S * eh: S * (eh + 1)], v_ps[st][:])

    # ---- per-head scores -> exp -> ctx/Z; normalize per head ----
    # mq for head h+1 is emitted BEFORE head h's ctx/z/normalize: DVE is
    # in-order, and the normalize ops wait on PE ctx completion — emitting
    # mq first keeps the next head's scores from stalling PE at each head
    # boundary.
    ctxn = mqp.tile([P, NB, S], fr, tag="ctxn", name=f"ctxn{it}", bufs=1)

    def emit_mq(h):
        mq = mqp.tile([P, NB, S], fr, tag="mq", name=f"mq{it}_{h}")
        for ic in range(NB):
            nc.vector.tensor_tensor(
                mq[:, ic, :], qT[:, ic, :],
                io["mixt_sb"][:, ic, h:h + 1].to_broadcast([P, S]),
                Alu.mult)
        return mq

    mq_next = emit_mq(0)
    for pair in range(NPAIR):
        for par in range(2):
            h = 2 * pair + par
            mq = mq_next
            w_tiles = []
            for st in range(4):
                sc = ps.tile([P, S], fp, tag="ps", name=f"sc{it}")
                for ic in range(NB):
                    nc.tensor.matmul(sc[:],
                                     lhsT=kT[:, ic, P * st: P * (st + 1)],
                                     rhs=mq[:, ic, :],
                                     start=(ic == 0), stop=(ic == NB - 1))
                wt = wp.tile([P, S], fr, tag="w", name=f"wt{it}")
                nc.scalar.activation(wt[:], sc[:], Act.Exp,
                                     bias=bias_sb[:, st, h:h + 1], scale=SCALE)
                w_tiles.append(wt)
            if h + 1 < H:
                mq_next = emit_mq(h + 1)
            # ctx: lhsT spans 128 v-columns so M=128 (fp32r needs full
            # weights); the head's real rows land at its row-half rh. z:
            # all-ones [s, 128] lhsT puts Z[u] in every output row.
            rh = DH * par
            ctx_h = ps.tile([P, S], fp, tag="ps", name=f"ctxh{it}")
            z_h = ps.tile([P, S], fp, tag="ps", name=f"zh{it}")
            for st in range(4):
                nc.tensor.matmul(ctx_h[:],
                                 lhsT=v_sb[:, st, DH * h - rh: DH * h - rh + P],
                                 rhs=w_tiles[st][:],
                                 start=(st == 0), stop=(st == 3))
                nc.tensor.matmul(z_h[:],
                                 lhsT=io["ones_sb"][:],
                                 rhs=w_tiles[st][:],
                                 start=(st == 0), stop=(st == 3))
            rz_sb = wkp.tile([P, S], fp, tag="wk", name=f"rz_sb{it}")
            nc.vector.reciprocal(rz_sb[rh:rh + DH, :], z_h[rh:rh + DH, :])
            nc.vector.tensor_tensor(ctxn[rh:rh + DH, pair, :],
                                    ctx_h[rh:rh + DH, :],
                                    rz_sb[rh:rh + DH, :], Alu.mult)
            nc.vector.tensor_scalar_add(
                ctxn[rh:rh + DH, pair, :], ctxn[rh:rh + DH, pair, :],
                io["bv_sb"][rh:rh + DH, pair:pair + 1])

    # ---- output projection (full, Wd streamed by contraction chunk) ----
    wdr = io["wd"].rearrange("(o p) m -> p o m", p=P)
    po = [ps.tile([P, S], fp, tag="ps", name=f"po{it}_{j}") for j in range(8)]
    for o in range(NB):
        wdo = wstream.tile([P, NB, P], fr, tag="wst", name=f"wdo{it}")
        wdo2 = wdo.rearrange("p a b -> p (a b)")
        nc.sync.dma_start(wdo2[:], wdr[:, o, :])
        for ut in range(4):
            for eh in range(2):
                nc.tensor.matmul(po[ut * 2 + eh][:],
                                 lhsT=ctxn[:, o, P * ut: P * (ut + 1)],
                                 rhs=wdo2[:, S * eh: S * (eh + 1)],
                                 start=(o == 0), stop=(o == NB - 1))

    # ---- epilogue: residual + bd, LayerNorm, full [512, 1024] output ----
    hidr = io["hid"].rearrange("(o p) m -> p o m", p=P)
    for ut in range(4):
        hid_t = ep.tile([P, D], fp, tag="hid", name=f"hid_t{it}")
        nc.sync.dma_start(hid_t[:], hidr[:, ut, :])
        r_sb = ep.tile([P, D], fp, tag="r", name=f"r_sb{it}")
        for eh in range(2):
            nc.vector.tensor_add(r_sb[:, S * eh: S * (eh + 1)],
                                 po[ut * 2 + eh][:],
                                 io["bd_b"][:, S * eh: S * (eh + 1)])
        nc.vector.tensor_add(r_sb[:], r_sb[:], hid_t[:])
        stats = ep.tile([P, 2, 6], fp, tag="stats", name=f"stats{it}")
        nc.vector.bn_stats(stats[:, 0, :], r_sb[:, 0:S])
        nc.vector.bn_stats(stats[:, 1, :], r_sb[:, S:D])
        mv = ep.tile([P, 2], fp, tag="mv", name=f"mv{it}")
        nc.vector.bn_aggr(mv[:], stats[:])
        std = ep.tile([P, 1], fp, tag="std", name=f"std{it}")
        nc.scalar.activation(std[:], mv[:, 1:2], Act.Sqrt,
                             bias=io["eps_t"][:], scale=1.0)
        nc.vector.reciprocal(std[:], std[:])
        nc.vector.tensor_scalar(out=r_sb[:], in0=r_sb[:],
                                scalar1=mv[:, 0:1], scalar2=std[:],
                                op0=Alu.subtract, op1=Alu.mult)
        nc.vector.tensor_tensor(r_sb[:], r_sb[:], io["gam_b"][:], Alu.mult)
        nc.vector.tensor_add(r_sb[:], r_sb[:], io["bet_b"][:])
        nc.sync.dma_start(io["out"][P * ut: P * (ut + 1), :], r_sb[:])


def _build(iters=1):
    import concourse.bass as bass
    import concourse.mybir as mybir
    import concourse.tile as tile
    from concourse import bacc

    fp = mybir.dt.float32
    fr = mybir.dt.float32r if F32R else fp

    nc = bacc.Bacc("TRN2", target_bir_lowering=False, debug=False,
                   num_devices=N_CORES)

    hiddenT = nc.dram_tensor("hiddenT", [D, S], fr, kind="ExternalInput").ap()
    hid = nc.dram_tensor("hid", [S, D], fp, kind="ExternalInput").ap()
    wq = nc.dram_tensor("wq", [D, D], fr, kind="ExternalInput").ap()
    wk = nc.dram_tensor("wk", [D, D], fr, kind="ExternalInput").ap()
    wv = nc.dram_tensor("wv", [D, D], fr, kind="ExternalInput").ap()
    wcb = nc.dram_tensor("wcb", [D, H], fr, kind="ExternalInput").ap()
    wd = nc.dram_tensor("wd", [D, D], fr, kind="ExternalInput").ap()
    mixt = nc.dram_tensor("mixt", [D, H], fr, kind="ExternalInput").ap()
    bvc = nc.dram_tensor("bvc", [D], fp, kind="ExternalInput").ap()
    lncnt = nc.dram_tensor("lncnt", [S], fp, kind="ExternalInput").ap()
    bd = nc.dram_tensor("bd", [D], fp, kind="ExternalInput").ap()
    gamma = nc.dram_tensor("gamma", [D], fp, kind="ExternalInput").ap()
    beta = nc.dram_tensor("beta", [D], fp, kind="ExternalInput").ap()
    out = nc.dram_tensor("out", [S, D], fp, kind="ExternalOutput").ap()

    def bcast_dram(vec_ap, parts):
        # DMA-replicate a [n] DRAM vector across `parts` partitions.
        return bass.AP(tensor=vec_ap.tensor, offset=vec_ap.offset,
                       ap=[[0, parts]] + [list(d) for d in vec_ap.ap])

    with tile.TileContext(nc) as tc:
        with (
            tc.tile_pool(name="singles", bufs=1) as singles,
            tc.tile_pool(name="mqp", bufs=2) as mqp,
            tc.tile_pool(name="wp", bufs=8) as wp,
            tc.tile_pool(name="wkp", bufs=4) as wkp,
            tc.tile_pool(name="ep", bufs=2) as ep,
            tc.tile_pool(name="wstream", bufs=4) as wstream,
            tc.tile_pool(name="ps", bufs=8, space="PSUM") as ps,
        ):
            pools = {"singles": singles, "mqp": mqp, "wp": wp, "wkp": wkp,
                     "ep": ep, "ps": ps, "wstream": wstream}
            # ---- constant / input loads (once) ----
            hT = singles.tile([P, NB, S], fr)
            hTr = hiddenT.rearrange("(o p) u -> p o u", p=P)
            for _o in range(NB):
                nc.sync.dma_start(hT[:, _o, :], hTr[:, _o, :])
            wcb_sb = singles.tile([P, NB, H], fr)
            nc.sync.dma_start(wcb_sb[:], wcb.rearrange("(o p) h -> p o h", p=P))
            mixt_sb = singles.tile([P, NB, H], fr)
            nc.sync.dma_start(mixt_sb[:], mixt.rearrange("(o p) h -> p o h", p=P))
            bv_sb = singles.tile([P, NB], fp)
            nc.sync.dma_start(bv_sb[:], bvc.rearrange("(o p) -> p o", p=P))
            lncnt_sb = singles.tile([P, 4], fp)
            nc.sync.dma_start(lncnt_sb[:], lncnt.rearrange("(o p) -> p o", p=P))
            bd_b = singles.tile([P, D], fp)
            nc.gpsimd.dma_start(out=bd_b[:], in_=bcast_dram(bd, P))
            gam_b = singles.tile([P, D], fp)
            nc.gpsimd.dma_start(out=gam_b[:], in_=bcast_dram(gamma, P))
            bet_b = singles.tile([P, D], fp)
            nc.gpsimd.dma_start(out=bet_b[:], in_=bcast_dram(beta, P))
            ones_f32 = singles.tile([P, P], fp)
            nc.vector.memset(ones_f32[:], 1.0)
            ones_sb = singles.tile([P, P], fr)
            with nc.allow_low_precision(reason="exact 1.0 constants to fp32r"):
                nc.vector.tensor_copy(ones_sb[:], ones_f32[:])
            eps_t = singles.tile([P, 1], fp)
            nc.vector.memset(eps_t[:], LN_EPS)

            io = {"hT": hT, "wq": wq, "wk": wk, "wv": wv, "wd": wd,
                  "wcb_sb": wcb_sb, "mixt_sb": mixt_sb, "bv_sb": bv_sb,
                  "lncnt_sb": lncnt_sb, "hid": hid, "bd_b": bd_b,
                  "gam_b": gam_b, "bet_b": bet_b, "ones_sb": ones_sb,
                  "eps_t": eps_t, "out": out}

            for it in range(iters):
                _emit(nc, tc, pools, io, it)

    nc.compile()
    return nc


def _get_nc(iters=1):
    key = ("nc", iters)
    if key not in _CACHE:
        _CACHE[key] = _build(iters)
    return _CACHE[key]


def _prepare_in_maps(hidden_states, fpos, tpos, Wq, Wk, Wcb, Wv, bv, mixing,
                     Wd, bd, ln_gamma, ln_beta):
    hs = np.ascontiguousarray(np.asarray(hidden_states, dtype=np.float32))
    tidx = np.asarray(tpos).astype(np.int64) % S
    counts = np.bincount(tidx, minlength=S).astype(np.float64)
    lncnt = np.where(counts > 0, np.log(np.maximum(counts, 1e-30)),
                     NEG_BIG).astype(np.float32)
    one = {
        "hiddenT": np.ascontiguousarray(hs.T),
        "hid": hs,
        "wq": np.ascontiguousarray(np.asarray(Wq, np.float32)),
        "wk": np.ascontiguousarray(np.asarray(Wk, np.float32)),
        "wv": np.ascontiguousarray(np.asarray(Wv, np.float32)),
        "wcb": np.ascontiguousarray(np.asarray(Wcb, np.float32)),
        "wd": np.ascontiguousarray(np.asarray(Wd, np.float32)),
        "mixt": np.ascontiguousarray(np.asarray(mixing, np.float32).T),
        "bvc": np.ascontiguousarray(np.asarray(bv, np.float32)),
        "lncnt": lncnt,
        "bd": np.ascontiguousarray(np.asarray(bd, np.float32)),
        "gamma": np.ascontiguousarray(np.asarray(ln_gamma, np.float32)),
        "beta": np.ascontiguousarray(np.asarray(ln_beta, np.float32)),
    }
    return [dict(one) for _ in range(N_CORES)]


def _run(inputs, trace=False, iters=1):
    from concourse import bass_utils
    nc = _get_nc(iters)
    in_maps = _prepare_in_maps(**inputs)
    res = bass_utils.run_bass_kernel_spmd(
        nc, in_maps, core_ids=list(range(N_CORES)), trace=trace)
    normedfull = res.results[0]["out"]
    fidx = np.asarray(inputs["fpos"]).astype(np.int64) % S
    return np.ascontiguousarray(normedfull[fidx]), res


def kernel(**inputs) -> np.ndarray:
    out, _ = _run(inputs, trace=False)
    return out



# revision 5
# speedup vs baseline: 2.4808x; 2.4808x over previous
"""Trainium2 Bass kernel for CollaborativeAttention (row-sharded, bf16).

Math: with S=512 unique positions and F=T=2048 gathered via fpos/tpos (mod 512),
the whole block collapses to the unique-position problem:
    qf = hs @ Wq ; kf = hs @ Wk ; vf = hs @ Wv + bv ; cbf = hs @ Wcb       [512, *]
    per head h:  w[u, s] = counts[s] * exp(scale*(qf[u]*mix[h]) . kf[s]
                                           + scale*cbf[s, h])
    ctx[u, h*64:(h+1)*64] = (w @ vf[:, h*64:(h+1)*64]) / w.sum(axis=1)
    outfull = ctx @ Wd + bd ; resfull = hs + outfull ; LN  -> normedfull   [512, 1024]
    output  = normedfull[fpos % 512]                                       [2048, 1024]
counts[s] = multiplicity of s in (tpos % 512); softmax over the 2048 keys is
exactly the count-weighted softmax over the 512 unique keys.

Distribution: collectives on this stack are far too slow (~ms), so the kernel
uses a zero-collective row shard: core c owns query rows 64c..64c+63. The k/v
projections (needed in full by every core) are replicated; everything else
(q, scores, softmax, context, output dense, LayerNorm) is 1/8 per core. The
host concatenates the 8 disjoint row blocks and applies the fpos gather.

Precision: the residual path keeps the attention output at ~2% of the signal,
so the whole attention path runs in bf16 (fp32 PSUM accumulation); the
residual + LayerNorm stay fp32. All weights are SBUF-resident in bf16.
"""

import math
import numpy as np

P = 128
S = 512
D = 1024
H = 16
DH = 64
Q = 64               # query rows per core
NB = D // P          # 8 contraction chunks
NKT = S // P         # 4 key tiles
NPAIR = H // 2       # 8 head pairs
NPP = NPAIR // 2     # 4 pair-pairs (2 pairs share a psum slot)
WCB = D + H          # packed [Wv | Wcb] columns
N_CORES = 8
SCALE = 1.0 / math.sqrt(D / H)  # 0.125
LN_EPS = 1e-5
NEG_BIG = -30000.0

_CACHE = {}


def _emit(nc, tc, pools, io, it):
    """Emit one full compute iteration (everything after the constant loads)."""
    import concourse.mybir as mybir

    fp = mybir.dt.float32
    bf = mybir.dt.bfloat16
    Alu = mybir.AluOpType
    Act = mybir.ActivationFunctionType

    acts, wp, ps = (pools[k] for k in ("acts", "wp", "ps"))
    hT = io["hT"]

    # ---- q projection (local 64 rows): q_sb [Q, D] ----
    q_sb = acts.tile([Q, D], bf, tag="q", name=f"q{it}")
    for eh in range(2):
        pq = ps.tile([Q, S], fp, tag="ps", name=f"pq{it}")
        for o in range(NB):
            nc.tensor.matmul(pq[:], lhsT=io["hTq"][:, o, :],
                             rhs=io["wq"][:, o, S * eh: S * (eh + 1)],
                             start=(o == 0), stop=(o == NB - 1))
        nc.scalar.copy(q_sb[:, S * eh: S * (eh + 1)], pq[:])

    # qT via PE transpose: qTp[:, o, :] = q_sb[:, 128o:128(o+1)]^T
    qTp = ps.tile([P, NB, Q], bf, tag="ps", name=f"qTp{it}")
    for o in range(NB):
        nc.tensor.transpose(qTp[:, o, :], q_sb[:, P * o: P * (o + 1)],
                            io["id64"][:])
    # mq[d, pair, hh, q] = qT[d, q] * mixing[h, d]  (DVE, per-partition scalar)
    mq = pools["mqp"].tile([P, NB, NPAIR, 2, Q], bf, tag="mq", name=f"mq{it}")
    for o in range(NB):
        for h in range(H):
            nc.vector.tensor_scalar(
                out=mq[:, o, h // 2, h % 2, :], in0=qTp[:, o, :],
                scalar1=io["mixt_sb"][:, o, h:h + 1], scalar2=None,
                op0=Alu.mult)

    # ---- k projection (full, replicated): kT_sb [d, keys] ----
    kT = acts.tile([P, NB, S], bf, tag="kT", name=f"kT{it}")
    for j in range(NB):
        pk = ps.tile([P, S], fp, tag="ps", name=f"pk{it}")
        for o in range(NB):
            nc.tensor.matmul(pk[:], lhsT=io["wk"][:, o, P * j: P * (j + 1)],
                             rhs=hT[:, o, :],
                             start=(o == 0), stop=(o == NB - 1))
        nc.scalar.copy(kT[:, j, :], pk[:])

    # ---- v (+ content bias) projection (full, replicated) ----
    v_sb = acts.tile([P, NKT, D], bf, tag="v", name=f"v{it}")
    bias_sb = acts.tile([P, NKT, H], fp, tag="bias", name=f"bias{it}")
    pcb = ps.tile([P, NKT, H], fp, tag="ps", name=f"pcb{it}")
    for kt in range(NKT):
        pv = [ps.tile([P, S], fp, tag="ps", name=f"pv{it}_{kt}_{eh}")
              for eh in range(2)]
        for o in range(NB):
            lhs = hT[:, o, P * kt: P * (kt + 1)]
            for eh in range(2):
                nc.tensor.matmul(pv[eh][:], lhsT=lhs,
                                 rhs=io["wvcb"][:, o, S * eh: S * (eh + 1)],
                                 start=(o == 0), stop=(o == NB - 1))
            nc.tensor.matmul(pcb[:, kt, :], lhsT=lhs,
                             rhs=io["wvcb"][:, o, D:WCB],
                             start=(o == 0), stop=(o == NB - 1),
                             skip_group_check=True)
        for eh in range(2):
            nc.scalar.copy(v_sb[:, kt, S * eh: S * (eh + 1)], pv[eh][:])
        # exp bias per key s and head: scale*cb[s, h] + ln(counts[s])
        nc.vector.scalar_tensor_tensor(
            out=bias_sb[:, kt, :], in0=pcb[:, kt, :], scalar=SCALE,
            in1=io["lncnt_sb"][:, kt:kt + 1].to_broadcast([P, H]),
            op0=Alu.mult, op1=Alu.add)

    # ---- scores -> exp -> ctx/z, processed in two key-halves ----
    # scores^T layout [keys, qpack]: per pair-pair pp, psum [P, pa, ktl, 128];
    # the exp bias (scale*cb + ln counts) is per key = per partition.
    w_t = [[None] * 2 for _ in range(NPAIR)]   # [pair][kh]
    sc_t = [[None] * 2 for _ in range(NPP)]    # [pp][kh]
    cz = [None] * NPP                          # ctx+z accum, [pp]

    for kh in range(2):
        for pp in range(NPP):
            sc_t[pp][kh] = ps.tile([P, 2, 2, P], fp, tag="ps",
                                   name=f"sc{it}_{kh}_{pp}")
        # start=True clears the whole psum bank's has_written state, so only
        # the first matmul touching each tile may set it; later regions'
        # first writes land via has_written=0 overwrite semantics.
        for o in range(NB):
            for ktl in range(2):
                kt = 2 * kh + ktl
                for pair in range(NPAIR):
                    nc.tensor.matmul(
                        sc_t[pair // 2][kh][:, pair % 2, ktl, :],
                        lhsT=kT[:, o, P * kt: P * (kt + 1)],
                        rhs=mq[:, o, pair],
                        start=(o == 0 and ktl == 0 and pair % 2 == 0),
                        stop=(o == NB - 1 and ktl == 1 and pair % 2 == 1),
                        skip_group_check=True)
    # exp (ACT): per (pair, kh, ktl, head-half); bias differs per head
    for kh in range(2):
        for pair in range(NPAIR):
            wt = wp.tile([P, 2, P], bf, tag="w", name=f"w{it}_{kh}_{pair}")
            w_t[pair][kh] = wt
            for ktl in range(2):
                kt = 2 * kh + ktl
                for hh in range(2):
                    nc.scalar.activation(
                        wt[:, ktl, DH * hh: DH * (hh + 1)],
                        sc_t[pair // 2][kh][:, pair % 2, ktl,
                                            DH * hh: DH * (hh + 1)],
                        Act.Exp,
                        bias=bias_sb[:, kt, 2 * pair + hh: 2 * pair + hh + 1],
                        scale=SCALE)
        # z and ctx accumulate across both key-halves
        if kh == 0:
            for pp in range(NPP):
                cz[pp] = ps.tile([P, 2, 2, P], fp, tag="ps",
                                 name=f"cz{it}_{pp}")
        for ktl in range(2):
            kt = 2 * kh + ktl
            for pair in range(NPAIR):   # z first: shared all-ones stationary
                nc.tensor.matmul(cz[pair // 2][:, pair % 2, 1, :],
                                 lhsT=io["ones_sb"][:],
                                 rhs=w_t[pair][kh][:, ktl, :],
                                 start=(kh == 0 and ktl == 0
                                        and pair % 2 == 0),
                                 stop=False,
                                 skip_group_check=True)
        for ktl in range(2):
            kt = 2 * kh + ktl
            for pair in range(NPAIR):
                nc.tensor.matmul(cz[pair // 2][:, pair % 2, 0, :],
                                 lhsT=v_sb[:, kt, P * pair: P * (pair + 1)],
                                 rhs=w_t[pair][kh][:, ktl, :],
                                 start=False,
                                 stop=(kh == 1 and ktl == 1
                                       and pair % 2 == 1),
                                 skip_group_check=True)

    # ---- normalize: ctxT[vcol, q] = ctx[vcol, q] / z[q] + bv[vcol] ----
    ctxT = acts.tile([P, NB, Q], bf, tag="ctxT", name=f"ctxT{it}")
    for pair in range(NPAIR):
        pp, pa = pair // 2, pair % 2
        rzb = acts.tile([P, P], fp, tag="rzb", name=f"rzb{it}_{pair}")
        nc.vector.reciprocal(rzb[:], cz[pp][:, pa, 1, :])
        for hh in range(2):
            r0 = DH * hh
            nc.vector.tensor_tensor(ctxT[r0:r0 + DH, pair, :],
                                    cz[pp][r0:r0 + DH, pa, 0, r0:r0 + DH],
                                    rzb[r0:r0 + DH, r0:r0 + DH], Alu.mult)
            nc.vector.tensor_scalar_add(ctxT[r0:r0 + DH, pair, :],
                                        ctxT[r0:r0 + DH, pair, :],
                                        io["bv_sb"][r0:r0 + DH,
                                                    pair:pair + 1])

    # ---- output projection ----
    po = [ps.tile([Q, S], fp, tag="ps", name=f"po{it}_{eh}")
          for eh in range(2)]
    for j in range(NB):
        for eh in range(2):
            nc.tensor.matmul(po[eh][:], lhsT=ctxT[:, j, :],
                             rhs=io["wd"][:, j, S * eh: S * (eh + 1)],
                             start=(j == 0), stop=(j == NB - 1))

    # ---- epilogue: residual + bd, LayerNorm, [64, 1024] row-block out ----
    r_sb = acts.tile([Q, D], fp, tag="r", name=f"r{it}")
    for eh in range(2):
        nc.vector.tensor_add(r_sb[:, S * eh: S * (eh + 1)], po[eh][:],
                             io["bd_b"][:, S * eh: S * (eh + 1)])
    nc.vector.tensor_add(r_sb[:], r_sb[:], io["hidq_sb"][:])
    stats = acts.tile([Q, 2, 6], fp, tag="stats", name=f"stats{it}")
    nc.vector.bn_stats(stats[:, 0, :], r_sb[:, 0:S])
    nc.vector.bn_stats(stats[:, 1, :], r_sb[:, S:D])
    mv = acts.tile([Q, 2], fp, tag="mv", name=f"mv{it}")
    nc.vector.bn_aggr(mv[:], stats[:])
    std = acts.tile([Q, 1], fp, tag="std", name=f"std{it}")
    nc.scalar.activation(std[:], mv[:, 1:2], Act.Sqrt,
                         bias=io["eps_t"][0:Q, :], scale=1.0)
    nc.vector.reciprocal(std[:], std[:])
    nc.vector.tensor_scalar(out=r_sb[:], in0=r_sb[:],
                            scalar1=mv[:, 0:1], scalar2=std[:],
                            op0=Alu.subtract, op1=Alu.mult)
    nc.vector.tensor_tensor(r_sb[:], r_sb[:], io["gam_b"][:], Alu.mult)
    nc.vector.tensor_add(r_sb[:], r_sb[:], io["bet_b"][:])
    nc.sync.dma_start(io["out"][:], r_sb[:])


def _build(iters=1):
    import concourse.bass as bass
    import concourse.mybir as mybir
    import concourse.tile as tile
    from concourse import bacc

    fp = mybir.dt.float32
    bf = mybir.dt.bfloat16

    nc = bacc.Bacc("TRN2", target_bir_lowering=False, debug=False,
                   num_devices=N_CORES)

    hTd = nc.dram_tensor("hT", [D, S], bf, kind="ExternalInput").ap()
    hTqd = nc.dram_tensor("hTq", [D, Q], bf, kind="ExternalInput").ap()
    hidqd = nc.dram_tensor("hidq", [Q, D], fp, kind="ExternalInput").ap()
    wkd = nc.dram_tensor("wk", [D, D], bf, kind="ExternalInput").ap()
    wqd = nc.dram_tensor("wq", [D, D], bf, kind="ExternalInput").ap()
    wvcbd = nc.dram_tensor("wvcb", [D, WCB], bf, kind="ExternalInput").ap()
    wdd = nc.dram_tensor("wd", [D, D], bf, kind="ExternalInput").ap()
    mixtd = nc.dram_tensor("mixt", [D, H], fp, kind="ExternalInput").ap()
    lncntd = nc.dram_tensor("lncnt", [S], fp, kind="ExternalInput").ap()
    bvd = nc.dram_tensor("bvc", [D], fp, kind="ExternalInput").ap()
    bdd = nc.dram_tensor("bd", [D], fp, kind="ExternalInput").ap()
    gammad = nc.dram_tensor("gamma", [D], fp, kind="ExternalInput").ap()
    betad = nc.dram_tensor("beta", [D], fp, kind="ExternalInput").ap()
    identd = nc.dram_tensor("ident", [Q, Q], bf, kind="ExternalInput").ap()
    outd = nc.dram_tensor("out", [Q, D], fp, kind="ExternalOutput").ap()

    def bcast_dram(vec_ap, parts):
        # DMA-replicate a [n] DRAM vector across `parts` partitions.
        return bass.AP(tensor=vec_ap.tensor, offset=vec_ap.offset,
                       ap=[[0, parts]] + [list(d) for d in vec_ap.ap])

    with tile.TileContext(nc) as tc:
        with (
            tc.tile_pool(name="singles", bufs=1) as singles,
            tc.tile_pool(name="acts", bufs=2) as acts,
            tc.tile_pool(name="mqp", bufs=2) as mqp,
            tc.tile_pool(name="wp", bufs=16) as wp,
            tc.tile_pool(name="ps", bufs=8, space="PSUM") as ps,
        ):
            pools = {"singles": singles, "acts": acts, "mqp": mqp,
                     "wp": wp, "ps": ps}
            # ---- constant / input loads (once) ----
            hT = singles.tile([P, NB, S], bf)
            hTr = hTd.rearrange("(o p) u -> p o u", p=P)
            for _o in range(NB):
                nc.sync.dma_start(hT[:, _o, :], hTr[:, _o, :])
            hTq = singles.tile([P, NB, Q], bf)
            nc.sync.dma_start(hTq[:], hTqd.rearrange("(o p) q -> p o q", p=P))
            wk_sb = singles.tile([P, NB, D], bf)
            wq_sb = singles.tile([P, NB, D], bf)
            wd_sb = singles.tile([P, NB, D], bf)
            for wdram, dest in ((wkd, wk_sb), (wqd, wq_sb), (wdd, wd_sb)):
                wr = wdram.rearrange("(o p) m -> p o m", p=P)
                for _o in range(NB):
                    nc.sync.dma_start(dest[:, _o, :], wr[:, _o, :])
            wvcb_sb = singles.tile([P, NB, WCB], bf)
            wvr = wvcbd.rearrange("(o p) m -> p o m", p=P)
            for _o in range(NB):
                nc.sync.dma_start(wvcb_sb[:, _o, :], wvr[:, _o, :])
            mixt_sb = singles.tile([P, NB, H], fp)
            nc.sync.dma_start(mixt_sb[:],
                              mixtd.rearrange("(o p) h -> p o h", p=P))
            lncnt_sb = singles.tile([P, NKT], fp)
            nc.sync.dma_start(lncnt_sb[:],
                              lncntd.rearrange("(a p) -> p a", p=P))
            bv_sb = singles.tile([P, NPAIR], fp)
            nc.sync.dma_start(bv_sb[:], bvd.rearrange("(a p) -> p a", p=P))
            hidq_sb = singles.tile([Q, D], fp)
            nc.sync.dma_start(hidq_sb[:], hidqd)
            id64 = singles.tile([Q, Q], bf)
            nc.sync.dma_start(id64[:], identd)
            bd_b = singles.tile([Q, D], fp)
            nc.gpsimd.dma_start(out=bd_b[:], in_=bcast_dram(bdd, Q))
            gam_b = singles.tile([Q, D], fp)
            nc.gpsimd.dma_start(out=gam_b[:], in_=bcast_dram(gammad, Q))
            bet_b = singles.tile([Q, D], fp)
            nc.gpsimd.dma_start(out=bet_b[:], in_=bcast_dram(betad, Q))
            ones_f32 = singles.tile([P, P], fp)
            nc.vector.memset(ones_f32[:], 1.0)
            ones_sb = singles.tile([P, P], bf)
            with nc.allow_low_precision(reason="exact 1.0 constants to bf16"):
                nc.vector.tensor_copy(ones_sb[:], ones_f32[:])
            eps_t = singles.tile([P, 1], fp)
            nc.vector.memset(eps_t[:], LN_EPS)

            io = {"hT": hT, "hTq": hTq, "wk": wk_sb, "wq": wq_sb,
                  "wvcb": wvcb_sb, "wd": wd_sb, "mixt_sb": mixt_sb,
                  "lncnt_sb": lncnt_sb, "bv_sb": bv_sb, "hidq_sb": hidq_sb,
                  "id64": id64, "bd_b": bd_b, "gam_b": gam_b,
                  "bet_b": bet_b, "ones_sb": ones_sb, "eps_t": eps_t,
                  "out": outd}

            with nc.allow_low_precision(
                    reason="attention path tolerates bf16; residual+LN fp32"):
                for it in range(iters):
                    _emit(nc, tc, pools, io, it)

    nc.compile()
    return nc


def _get_nc(iters=1):
    key = ("nc", iters)
    if key not in _CACHE:
        _CACHE[key] = _build(iters)
    return _CACHE[key]


def _prepare_in_maps(hidden_states, fpos, tpos, Wq, Wk, Wcb, Wv, bv, mixing,
                     Wd, bd, ln_gamma, ln_beta):
    import ml_dtypes
    bf = ml_dtypes.bfloat16
    hs = np.ascontiguousarray(np.asarray(hidden_states, dtype=np.float32))
    tidx = np.asarray(tpos).astype(np.int64) % S
    counts = np.bincount(tidx, minlength=S).astype(np.float64)
    lncnt = np.where(counts > 0, np.log(np.maximum(counts, 1e-30)),
                     NEG_BIG).astype(np.float32)
    wvcb = np.concatenate([np.asarray(Wv, np.float32),
                           np.asarray(Wcb, np.float32)], axis=1)
    common = {
        "hT": np.ascontiguousarray(hs.T).astype(bf),
        "wk": np.ascontiguousarray(np.asarray(Wk, np.float32)).astype(bf),
        "wq": np.ascontiguousarray(np.asarray(Wq, np.float32)).astype(bf),
        "wvcb": np.ascontiguousarray(wvcb).astype(bf),
        "wd": np.ascontiguousarray(np.asarray(Wd, np.float32)).astype(bf),
        "mixt": np.ascontiguousarray(np.asarray(mixing, np.float32).T),
        "lncnt": lncnt,
        "bvc": np.ascontiguousarray(np.asarray(bv, np.float32)),
        "bd": np.ascontiguousarray(np.asarray(bd, np.float32)),
        "gamma": np.ascontiguousarray(np.asarray(ln_gamma, np.float32)),
        "beta": np.ascontiguousarray(np.asarray(ln_beta, np.float32)),
        "ident": np.eye(Q, dtype=np.float32).astype(bf),
    }
    in_maps = []
    for c in range(N_CORES):
        m = dict(common)
        rows = hs[Q * c: Q * (c + 1)]
        m["hTq"] = np.ascontiguousarray(rows.T).astype(bf)
        m["hidq"] = np.ascontiguousarray(rows)
        in_maps.append(m)
    return in_maps


def _run(inputs, trace=False, iters=1):
    from concourse import bass_utils
    nc = _get_nc(iters)
    in_maps = _prepare_in_maps(**inputs)
    res = bass_utils.run_bass_kernel_spmd(
        nc, in_maps, core_ids=list(range(N_CORES)), trace=trace)
    normedfull = np.concatenate(
        [np.asarray(res.results[c]["out"]) for c in range(N_CORES)], axis=0)
    fidx = np.asarray(inputs["fpos"]).astype(np.int64) % S
    return np.ascontiguousarray(normedfull[fidx]), res


def kernel(**inputs) -> np.ndarray:
    out, _ = _run(inputs, trace=False)
    return out


# revision 7
# speedup vs baseline: 3.8096x; 1.5356x over previous
"""Trainium2 Bass kernel for CollaborativeAttention (row-sharded, fp8).

Math: with S=512 unique positions and F=T=2048 gathered via fpos/tpos (mod 512),
the whole block collapses to the unique-position problem:
    qf = hs @ Wq ; kf = hs @ Wk ; vf = hs @ Wv + bv ; cbf = hs @ Wcb       [512, *]
    per head h:  w[u, s] = counts[s] * exp(scale*(qf[u]*mix[h]) . kf[s]
                                           + scale*cbf[s, h])
    ctx[u, h*64:(h+1)*64] = (w @ vf[:, h*64:(h+1)*64]) / w.sum(axis=1)
    outfull = ctx @ Wd + bd ; resfull = hs + outfull ; LN  -> normedfull   [512, 1024]
    output  = normedfull[fpos % 512]                                       [2048, 1024]
counts[s] = multiplicity of s in (tpos % 512); softmax over the 2048 keys is
exactly the count-weighted softmax over the 512 unique keys.

Distribution: collectives on this stack are far too slow (~ms), so the kernel
uses a zero-collective row shard: core c owns query rows 64c..64c+63. The k/v
projections (needed in full by every core) are replicated; everything else
(q, scores, softmax, context, output dense, LayerNorm) is 1/8 per core. The
host concatenates the 8 disjoint row blocks and applies the fpos gather.

Precision: the residual path keeps the attention output at ~2% of the signal,
so the whole attention path runs in fp8 e4m3 with DoubleRow matmuls (fp32
PSUM accumulation); the residual + LayerNorm stay fp32. Power-of-2 scale
factors keep fp8 operands in the normal range: weights are pre-scaled x128 on
the host (compensated at PSUM eviction), mixing x8 (compensated in the exp
scale), the softmax reciprocal is computed as 32/z (all-ones lhsT holds 1/32)
so the stored context is x32 (compensated with bv x32 and a /4096 at the
output-dense eviction). All weights are SBUF-resident.
"""

import math
import numpy as np

P = 128
S = 512
D = 1024
H = 16
DH = 64
Q = 64               # query rows per core
NB = D // P          # 8 contraction chunks
ND = NB // 2         # 4 DoubleRow double-chunks
NKT = S // P         # 4 key tiles
NPAIR = H // 2       # 8 head pairs
NPP = NPAIR // 2     # 4 pair-pairs (2 pairs share a psum slot)
WCB = D + H          # packed [Wv | Wcb] columns
N_CORES = 8
SCALE = 1.0 / math.sqrt(D / H)  # 0.125
LN_EPS = 1e-5
NEG_BIG = -30000.0

WSCL = 128.0         # host weight scale (power of 2, keeps fp8 normal-range)
MSCL = 8.0           # host mixing scale
ZSCL = 32.0          # context scale via ones=1/ZSCL z-reduction

_CACHE = {}


def _emit(nc, tc, pools, io, it):
    """Emit one full compute iteration (everything after the constant loads)."""
    import concourse.mybir as mybir

    fp = mybir.dt.float32
    f8 = mybir.dt.float8e4
    bf = mybir.dt.bfloat16
    Alu = mybir.AluOpType
    Act = mybir.ActivationFunctionType
    DR = mybir.MatmulPerfMode.DoubleRow

    acts, wp, ps = (pools[k] for k in ("acts", "wp", "ps"))
    hT = io["hT"]

    # ---- q projection (local 64 rows): q_sb [Q, D], true scale ----
    q_sb = acts.tile([Q, D], bf, tag="q", name=f"q{it}")
    for eh in range(2):
        pq = ps.tile([Q, S], fp, tag="ps", name=f"pq{it}")
        for oo in range(ND):
            nc.tensor.matmul(pq[:], lhsT=io["hTq"][:, 2 * oo: 2 * oo + 2, :],
                             rhs=io["wq"][:, 2 * oo: 2 * oo + 2,
                                          S * eh: S * (eh + 1)],
                             start=(oo == 0), stop=(oo == ND - 1),
                             perf_mode=DR)
        nc.scalar.activation(q_sb[:, S * eh: S * (eh + 1)], pq[:],
                             Act.Copy, scale=1.0 / WSCL)

    # qT via PE transpose: qTp[:, o, :] = q_sb[:, 128o:128(o+1)]^T
    qTp = ps.tile([P, NB, Q], bf, tag="ps", name=f"qTp{it}")
    for o in range(NB):
        nc.tensor.transpose(qTp[:, o, :], q_sb[:, P * o: P * (o + 1)],
                            io["id64"][:])
    # mq[d, pair, hh, q] = qT[d, q] * (8*mixing[h, d])  (DVE per-part scalar)
    mq = pools["mqp"].tile([P, NB, NPAIR, 2, Q], f8, tag="mq", name=f"mq{it}")
    for o in range(NB):
        for h in range(H):
            nc.vector.tensor_scalar(
                out=mq[:, o, h // 2, h % 2, :], in0=qTp[:, o, :],
                scalar1=io["mixt_sb"][:, o, h:h + 1], scalar2=None,
                op0=Alu.mult)

    # ---- k projection (full, replicated): kT_sb [d, keys], true scale ----
    kT = acts.tile([P, NB, S], f8, tag="kT", name=f"kT{it}")
    for j in range(NB):
        pk = ps.tile([P, S], fp, tag="ps", name=f"pk{it}")
        for oo in range(ND):
            nc.tensor.matmul(pk[:],
                             lhsT=io["wk"][:, 2 * oo: 2 * oo + 2,
                                           P * j: P * (j + 1)],
                             rhs=hT[:, 2 * oo: 2 * oo + 2, :],
                             start=(oo == 0), stop=(oo == ND - 1),
                             perf_mode=DR)
        nc.scalar.activation(kT[:, j, :], pk[:], Act.Copy, scale=1.0 / WSCL)

    # ---- v (+ content bias) projection (full, replicated) ----
    v_sb = acts.tile([P, NKT, D], f8, tag="v", name=f"v{it}")
    bias_sb = acts.tile([P, NKT, H], fp, tag="bias", name=f"bias{it}")
    pcb = ps.tile([P, NKT, H], fp, tag="ps", name=f"pcb{it}")
    for kt in range(NKT):
        pv = [ps.tile([P, S], fp, tag="ps", name=f"pv{it}_{kt}_{eh}")
              for eh in range(2)]
        for oo in range(ND):
            lhs = hT[:, 2 * oo: 2 * oo + 2, P * kt: P * (kt + 1)]
            for eh in range(2):
                nc.tensor.matmul(pv[eh][:], lhsT=lhs,
                                 rhs=io["wvcb"][:, 2 * oo: 2 * oo + 2,
                                                S * eh: S * (eh + 1)],
                                 start=(oo == 0), stop=(oo == ND - 1),
                                 perf_mode=DR)
            nc.tensor.matmul(pcb[:, kt, :], lhsT=lhs,
                             rhs=io["wvcb"][:, 2 * oo: 2 * oo + 2, D:WCB],
                             start=(oo == 0), stop=(oo == ND - 1),
                             perf_mode=DR, skip_group_check=True)
        for eh in range(2):
            nc.scalar.activation(v_sb[:, kt, S * eh: S * (eh + 1)], pv[eh][:],
                                 Act.Copy, scale=1.0 / WSCL)
        # exp bias per key s and head: scale*cb[s, h] + ln(counts[s])
        nc.vector.scalar_tensor_tensor(
            out=bias_sb[:, kt, :], in0=pcb[:, kt, :], scalar=SCALE / WSCL,
            in1=io["lncnt_sb"][:, kt:kt + 1].to_broadcast([P, H]),
            op0=Alu.mult, op1=Alu.add)

    # ---- scores -> exp -> ctx/z, processed in two key-halves ----
    # scores^T layout [keys, qpack]: per pair-pair pp, psum [P, pa, ktl, 128];
    # the exp bias (scale*cb + ln counts) is per key = per partition.
    w_t = [[None] * 2 for _ in range(NPAIR)]   # [pair][kh]
    sc_t = [[None] * 2 for _ in range(NPP)]    # [pp][kh]
    cz = [None] * NPP                          # ctx+z accum, [pp]

    for kh in range(2):
        for pp in range(NPP):
            sc_t[pp][kh] = ps.tile([P, 2, 2, P], fp, tag="ps",
                                   name=f"sc{it}_{kh}_{pp}")
        # start=True clears the whole psum bank's has_written state, so only
        # the first matmul touching each tile may set it; later regions'
        # first writes land via has_written=0 overwrite semantics.
        for oo in range(ND):
            for ktl in range(2):
                kt = 2 * kh + ktl
                for pair in range(NPAIR):
                    nc.tensor.matmul(
                        sc_t[pair // 2][kh][:, pair % 2, ktl, :],
                        lhsT=kT[:, 2 * oo: 2 * oo + 2, P * kt: P * (kt + 1)],
                        rhs=mq[:, 2 * oo: 2 * oo + 2, pair],
                        start=(oo == 0 and ktl == 0 and pair % 2 == 0),
                        stop=(oo == ND - 1 and ktl == 1 and pair % 2 == 1),
                        perf_mode=DR, skip_group_check=True)
    # exp (ACT): per (pair, kh, ktl, head-half); bias differs per head.
    # psum holds 8*sc_true (mixing x8), so the activation scale is SCALE/8.
    for kh in range(2):
        for pair in range(NPAIR):
            wt = wp.tile([P, 2, P], f8, tag="w", name=f"w{it}_{kh}_{pair}")
            w_t[pair][kh] = wt
            for ktl in range(2):
                kt = 2 * kh + ktl
                for hh in range(2):
                    nc.scalar.activation(
                        wt[:, ktl, DH * hh: DH * (hh + 1)],
                        sc_t[pair // 2][kh][:, pair % 2, ktl,
                                            DH * hh: DH * (hh + 1)],
                        Act.Exp,
                        bias=bias_sb[:, kt, 2 * pair + hh: 2 * pair + hh + 1],
                        scale=SCALE / MSCL)
        # z and ctx accumulate across both key-halves
        if kh == 0:
            for pp in range(NPP):
                cz[pp] = ps.tile([P, 2, 2, P], fp, tag="ps",
                                 name=f"cz{it}_{pp}")
        for ktl in range(2):
            kt = 2 * kh + ktl
            for pair in range(NPAIR):   # z first: shared 1/32-valued lhsT
                nc.tensor.matmul(cz[pair // 2][:, pair % 2, 1, :],
                                 lhsT=io["ones_sb"][:],
                                 rhs=w_t[pair][kh][:, ktl, :],
                                 start=(kh == 0 and ktl == 0
                                        and pair % 2 == 0),
                                 stop=False,
                                 skip_group_check=True)
        for ktl in range(2):
            kt = 2 * kh + ktl
            for pair in range(NPAIR):
                nc.tensor.matmul(cz[pair // 2][:, pair % 2, 0, :],
                                 lhsT=v_sb[:, kt, P * pair: P * (pair + 1)],
                                 rhs=w_t[pair][kh][:, ktl, :],
                                 start=False,
                                 stop=(kh == 1 and ktl == 1
                                       and pair % 2 == 1),
                                 skip_group_check=True)

    # ---- normalize: ctxT[vcol, q] = 32*(ctx[vcol, q]/z[q]) + 32*bv[vcol] ---
    # z psum holds z/32, so its reciprocal is 32/z directly.
    ctxT = acts.tile([P, NB, Q], f8, tag="ctxT", name=f"ctxT{it}")
    for pair in range(NPAIR):
        pp, pa = pair // 2, pair % 2
        rzb = acts.tile([P, P], fp, tag="rzb", name=f"rzb{it}_{pair}")
        nc.vector.reciprocal(rzb[:], cz[pp][:, pa, 1, :])
        for hh in range(2):
            r0 = DH * hh
            nc.vector.tensor_tensor(ctxT[r0:r0 + DH, pair, :],
                                    cz[pp][r0:r0 + DH, pa, 0, r0:r0 + DH],
                                    rzb[r0:r0 + DH, r0:r0 + DH], Alu.mult)
            nc.vector.tensor_scalar_add(ctxT[r0:r0 + DH, pair, :],
                                        ctxT[r0:r0 + DH, pair, :],
                                        io["bv_sb"][r0:r0 + DH,
                                                    pair:pair + 1])

    # ---- output projection (psum = 32*ctx_n @ 128*Wd = 4096*out) ----
    po = [ps.tile([Q, S], fp, tag="ps", name=f"po{it}_{eh}")
          for eh in range(2)]
    for oo in range(ND):
        for eh in range(2):
            nc.tensor.matmul(po[eh][:],
                             lhsT=ctxT[:, 2 * oo: 2 * oo + 2, :],
                             rhs=io["wd"][:, 2 * oo: 2 * oo + 2,
                                          S * eh: S * (eh + 1)],
                             start=(oo == 0), stop=(oo == ND - 1),
                             perf_mode=DR)

    # ---- epilogue: residual + bd, LayerNorm, [64, 1024] row-block out ----
    r_sb = acts.tile([Q, D], fp, tag="r", name=f"r{it}")
    for eh in range(2):
        nc.vector.scalar_tensor_tensor(
            out=r_sb[:, S * eh: S * (eh + 1)], in0=po[eh][:],
            scalar=1.0 / (WSCL * ZSCL),
            in1=io["bd_b"][:, S * eh: S * (eh + 1)],
            op0=Alu.mult, op1=Alu.add)
    nc.vector.tensor_add(r_sb[:], r_sb[:], io["hidq_sb"][:])
    stats = acts.tile([Q, 2, 6], fp, tag="stats", name=f"stats{it}")
    nc.vector.bn_stats(stats[:, 0, :], r_sb[:, 0:S])
    nc.vector.bn_stats(stats[:, 1, :], r_sb[:, S:D])
    mv = acts.tile([Q, 2], fp, tag="mv", name=f"mv{it}")
    nc.vector.bn_aggr(mv[:], stats[:])
    std = acts.tile([Q, 1], fp, tag="std", name=f"std{it}")
    nc.scalar.activation(std[:], mv[:, 1:2], Act.Sqrt,
                         bias=io["eps_t"][0:Q, :], scale=1.0)
    nc.vector.reciprocal(std[:], std[:])
    nc.vector.tensor_scalar(out=r_sb[:], in0=r_sb[:],
                            scalar1=mv[:, 0:1], scalar2=std[:],
                            op0=Alu.subtract, op1=Alu.mult)
    nc.vector.tensor_tensor(r_sb[:], r_sb[:], io["gam_b"][:], Alu.mult)
    nc.vector.tensor_add(r_sb[:], r_sb[:], io["bet_b"][:])
    nc.sync.dma_start(io["out"][:], r_sb[:])


def _build(iters=1):
    import concourse.bass as bass
    import concourse.mybir as mybir
    import concourse.tile as tile
    from concourse import bacc

    fp = mybir.dt.float32
    f8 = mybir.dt.float8e4

    nc = bacc.Bacc("TRN2", target_bir_lowering=False, debug=False,
                   num_devices=N_CORES)

    hTd = nc.dram_tensor("hT", [D, S], f8, kind="ExternalInput").ap()
    hTqd = nc.dram_tensor("hTq", [D, Q], f8, kind="ExternalInput").ap()
    hidqd = nc.dram_tensor("hidq", [Q, D], fp, kind="ExternalInput").ap()
    wkd = nc.dram_tensor("wk", [D, D], f8, kind="ExternalInput").ap()
    wqd = nc.dram_tensor("wq", [D, D], f8, kind="ExternalInput").ap()
    wvcbd = nc.dram_tensor("wvcb", [D, WCB], f8, kind="ExternalInput").ap()
    wdd = nc.dram_tensor("wd", [D, D], f8, kind="ExternalInput").ap()
    mixtd = nc.dram_tensor("mixt", [D, H], fp, kind="ExternalInput").ap()
    lncntd = nc.dram_tensor("lncnt", [S], fp, kind="ExternalInput").ap()
    bvd = nc.dram_tensor("bvc", [D], fp, kind="ExternalInput").ap()
    bdd = nc.dram_tensor("bd", [D], fp, kind="ExternalInput").ap()
    gammad = nc.dram_tensor("gamma", [D], fp, kind="ExternalInput").ap()
    betad = nc.dram_tensor("beta", [D], fp, kind="ExternalInput").ap()
    identd = nc.dram_tensor("ident", [Q, Q], mybir.dt.bfloat16, kind="ExternalInput").ap()
    outd = nc.dram_tensor("out", [Q, D], fp, kind="ExternalOutput").ap()

    def bcast_dram(vec_ap, parts):
        # DMA-replicate a [n] DRAM vector across `parts` partitions.
        return bass.AP(tensor=vec_ap.tensor, offset=vec_ap.offset,
                       ap=[[0, parts]] + [list(d) for d in vec_ap.ap])

    with tile.TileContext(nc) as tc:
        with (
            tc.tile_pool(name="singles", bufs=1) as singles,
            tc.tile_pool(name="acts", bufs=2) as acts,
            tc.tile_pool(name="mqp", bufs=2) as mqp,
            tc.tile_pool(name="wp", bufs=16) as wp,
            tc.tile_pool(name="ps", bufs=8, space="PSUM") as ps,
        ):
            pools = {"singles": singles, "acts": acts, "mqp": mqp,
                     "wp": wp, "ps": ps}
            # ---- constant / input loads (once) ----
            hT = singles.tile([P, NB, S], f8)
            hTr = hTd.rearrange("(o p) u -> p o u", p=P)
            for _o in range(NB):
                nc.sync.dma_start(hT[:, _o, :], hTr[:, _o, :])
            hTq = singles.tile([P, NB, Q], f8)
            nc.sync.dma_start(hTq[:], hTqd.rearrange("(o p) q -> p o q", p=P))
            wk_sb = singles.tile([P, NB, D], f8)
            wq_sb = singles.tile([P, NB, D], f8)
            wd_sb = singles.tile([P, NB, D], f8)
            for wdram, dest in ((wkd, wk_sb), (wqd, wq_sb), (wdd, wd_sb)):
                wr = wdram.rearrange("(o p) m -> p o m", p=P)
                for _o in range(NB):
                    nc.sync.dma_start(dest[:, _o, :], wr[:, _o, :])
            wvcb_sb = singles.tile([P, NB, WCB], f8)
            wvr = wvcbd.rearrange("(o p) m -> p o m", p=P)
            for _o in range(NB):
                nc.sync.dma_start(wvcb_sb[:, _o, :], wvr[:, _o, :])
            mixt_sb = singles.tile([P, NB, H], fp)
            nc.sync.dma_start(mixt_sb[:],
                              mixtd.rearrange("(o p) h -> p o h", p=P))
            lncnt_sb = singles.tile([P, NKT], fp)
            nc.sync.dma_start(lncnt_sb[:],
                              lncntd.rearrange("(a p) -> p a", p=P))
            bv_sb = singles.tile([P, NPAIR], fp)
            nc.sync.dma_start(bv_sb[:], bvd.rearrange("(a p) -> p a", p=P))
            hidq_sb = singles.tile([Q, D], fp)
            nc.sync.dma_start(hidq_sb[:], hidqd)
            id64 = singles.tile([Q, Q], mybir.dt.bfloat16)
            nc.sync.dma_start(id64[:], identd)
            bd_b = singles.tile([Q, D], fp)
            nc.gpsimd.dma_start(out=bd_b[:], in_=bcast_dram(bdd, Q))
            gam_b = singles.tile([Q, D], fp)
            nc.gpsimd.dma_start(out=gam_b[:], in_=bcast_dram(gammad, Q))
            bet_b = singles.tile([Q, D], fp)
            nc.gpsimd.dma_start(out=bet_b[:], in_=bcast_dram(betad, Q))
            ones_f32 = singles.tile([P, P], fp)
            nc.vector.memset(ones_f32[:], 1.0 / ZSCL)
            ones_sb = singles.tile([P, P], f8)
            with nc.allow_low_precision(reason="exact 1/32 constants to fp8"):
                nc.vector.tensor_copy(ones_sb[:], ones_f32[:])
            eps_t = singles.tile([P, 1], fp)
            nc.vector.memset(eps_t[:], LN_EPS)

            io = {"hT": hT, "hTq": hTq, "wk": wk_sb, "wq": wq_sb,
                  "wvcb": wvcb_sb, "wd": wd_sb, "mixt_sb": mixt_sb,
                  "lncnt_sb": lncnt_sb, "bv_sb": bv_sb, "hidq_sb": hidq_sb,
                  "id64": id64, "bd_b": bd_b, "gam_b": gam_b,
                  "bet_b": bet_b, "ones_sb": ones_sb, "eps_t": eps_t,
                  "out": outd}

            with nc.allow_low_precision(
                    reason="attention path tolerates fp8; residual+LN fp32"):
                for it in range(iters):
                    _emit(nc, tc, pools, io, it)

    nc.compile()
    return nc


def _get_nc(iters=1):
    key = ("nc", iters)
    if key not in _CACHE:
        _CACHE[key] = _build(iters)
    return _CACHE[key]


def _prepare_in_maps(hidden_states, fpos, tpos, Wq, Wk, Wcb, Wv, bv, mixing,
                     Wd, bd, ln_gamma, ln_beta):
    import ml_dtypes
    f8 = ml_dtypes.float8_e4m3
    hs = np.ascontiguousarray(np.asarray(hidden_states, dtype=np.float32))
    tidx = np.asarray(tpos).astype(np.int64) % S
    counts = np.bincount(tidx, minlength=S).astype(np.float64)
    lncnt = np.where(counts > 0, np.log(np.maximum(counts, 1e-30)),
                     NEG_BIG).astype(np.float32)
    wvcb = np.concatenate([np.asarray(Wv, np.float32),
                           np.asarray(Wcb, np.float32)], axis=1)
    common = {
        "hT": np.ascontiguousarray(hs.T).astype(f8),
        "wk": (WSCL * np.asarray(Wk, np.float32)).astype(f8),
        "wq": (WSCL * np.asarray(Wq, np.float32)).astype(f8),
        "wvcb": (WSCL * wvcb).astype(f8),
        "wd": (WSCL * np.asarray(Wd, np.float32)).astype(f8),
        "mixt": np.ascontiguousarray(
            MSCL * np.asarray(mixing, np.float32).T),
        "lncnt": lncnt,
        "bvc": ZSCL * np.ascontiguousarray(np.asarray(bv, np.float32)),
        "bd": np.ascontiguousarray(np.asarray(bd, np.float32)),
        "gamma": np.ascontiguousarray(np.asarray(ln_gamma, np.float32)),
        "beta": np.ascontiguousarray(np.asarray(ln_beta, np.float32)),
        "ident": np.eye(Q, dtype=np.float32).astype(ml_dtypes.bfloat16),
    }
    in_maps = []
    for c in range(N_CORES):
        m = dict(common)
        rows = hs[Q * c: Q * (c + 1)]
        m["hTq"] = np.ascontiguousarray(rows.T).astype(f8)
        m["hidq"] = np.ascontiguousarray(rows)
        in_maps.append(m)
    return in_maps


def _run(inputs, trace=False, iters=1):
    from concourse import bass_utils
    nc = _get_nc(iters)
    in_maps = _prepare_in_maps(**inputs)
    res = bass_utils.run_bass_kernel_spmd(
        nc, in_maps, core_ids=list(range(N_CORES)), trace=trace)
    normedfull = np.concatenate(
        [np.asarray(res.results[c]["out"]) for c in range(N_CORES)], axis=0)
    fidx = np.asarray(inputs["fpos"]).astype(np.int64) % S
    return np.ascontiguousarray(normedfull[fidx]), res


def kernel(**inputs) -> np.ndarray:
    out, _ = _run(inputs, trace=False)
    return out


# revision 12
# speedup vs baseline: 12.8609x; 3.3759x over previous
"""Trainium2 Bass kernel for CollaborativeAttention (row-sharded, fp8).

Math: with S=512 unique positions and F=T=2048 gathered via fpos/tpos (mod 512),
the whole block collapses to the unique-position problem:
    qf = hs @ Wq ; kf = hs @ Wk ; vf = hs @ Wv + bv ; cbf = hs @ Wcb       [512, *]
    per head h:  w[u, s] = counts[s] * exp(scale*(qf[u]*mix[h]) . kf[s]
                                           + scale*cbf[s, h])
    ctx[u, h*64:(h+1)*64] = (w @ vf[:, h*64:(h+1)*64]) / w.sum(axis=1)
    outfull = ctx @ Wd + bd ; resfull = hs + outfull ; LN  -> normedfull   [512, 1024]
    output  = normedfull[fpos % 512]                                       [2048, 1024]
counts[s] = multiplicity of s in (tpos % 512); softmax over the 2048 keys is
exactly the count-weighted softmax over the 512 unique keys.

Distribution: collectives on this stack are far too slow (~ms), so the kernel
uses a zero-collective row shard: core c owns query rows 64c..64c+63. The k/v
projections (needed in full by every core) are replicated; everything else
(q, scores, softmax, context, output dense, LayerNorm) is 1/8 per core. The
host concatenates the 8 disjoint row blocks and applies the fpos gather.

Precision: the residual path keeps the attention output at ~2% of the signal,
so the whole attention path runs in fp8 e4m3 with DoubleRow matmuls (fp32
PSUM accumulation); the residual + LayerNorm stay fp32. Power-of-2 scale
factors keep fp8 operands in the normal range: weights are pre-scaled x128 on
the host (compensated at PSUM eviction), mixing x8 (compensated in the exp
scale), the softmax reciprocal is computed as 32/z (all-ones lhsT holds 1/32)
so the stored context is x32 (compensated with bv x32 and a /4096 at the
output-dense eviction). All weights are SBUF-resident.
"""

import math
import numpy as np

P = 128
S = 512
D = 1024
H = 16
DH = 64
Q = 64               # query rows per core
NB = D // P          # 8 contraction chunks
ND = NB // 2         # 4 DoubleRow double-chunks
NKT = S // P         # 4 key tiles
NPAIR = H // 2       # 8 head pairs
NPP = NPAIR // 2     # 4 pair-pairs (2 pairs share a psum slot)
WCB = D + H          # packed [Wv | Wcb] columns
N_CORES = 8
SCALE = 1.0 / math.sqrt(D / H)  # 0.125
LN_EPS = 1e-5
NEG_BIG = -30000.0

WSCL = 128.0         # host weight scale (power of 2, keeps fp8 normal-range)
MSCL = 8.0           # host mixing scale
ZSCL = 32.0          # context scale via ones=1/ZSCL z-reduction

_CACHE = {}


def _emit(nc, tc, pools, io, it):
    """Emit one full compute iteration (everything after the constant loads)."""
    import concourse.mybir as mybir

    fp = mybir.dt.float32
    f8 = mybir.dt.float8e4
    bf = mybir.dt.bfloat16
    Alu = mybir.AluOpType
    Act = mybir.ActivationFunctionType
    DR = mybir.MatmulPerfMode.DoubleRow

    acts, wp, ps = (pools[k] for k in ("acts", "wp", "ps"))
    hT = io["hT"]

    # ---- q projection (local 64 rows): q_sb [Q, D], true scale ----
    q_sb = acts.tile([Q, D], bf, tag="q", name=f"q{it}")
    for eh in range(2):
        pq = ps.tile([Q, S], fp, tag="ps", name=f"pq{it}")
        for oo in range(ND):
            nc.tensor.matmul(pq[:], lhsT=io["hTq"][:, 2 * oo: 2 * oo + 2, :],
                             rhs=io["wq"][:, 2 * oo: 2 * oo + 2,
                                          S * eh: S * (eh + 1)],
                             start=(oo == 0), stop=(oo == ND - 1),
                             perf_mode=DR)
        nc.scalar.activation(q_sb[:, S * eh: S * (eh + 1)], pq[:],
                             Act.Copy, scale=1.0 / WSCL)

    # qT via PE transpose: qTp[:, o, :] = q_sb[:, 128o:128(o+1)]^T
    qTp = ps.tile([P, NB, Q], bf, tag="ps", name=f"qTp{it}")
    for o in range(NB):
        nc.tensor.transpose(qTp[:, o, :], q_sb[:, P * o: P * (o + 1)],
                            io["id64"][:])
    # one bulk psum->SBUF copy so the 128 mq ops read SBUF (58-cycle DVE
    # overhead) instead of PSUM (120-cycle)
    qT_sb = acts.tile([P, NB, Q], bf, tag="qT", name=f"qT{it}")
    nc.scalar.copy(qT_sb[:], qTp.rearrange("p a b -> p (a b)"))
    # mq[d, pair, hh, q] = qT[d, q] * (8*mixing[h, d])  (DVE per-part scalar)
    mq = pools["mqp"].tile([P, NB, NPAIR, 2, Q], f8, tag="mq", name=f"mq{it}")
    for o in range(NB):
        for h in range(H):
            nc.vector.tensor_scalar(
                out=mq[:, o, h // 2, h % 2, :], in0=qT_sb[:, o, :],
                scalar1=io["mixt_sb"][:, o, h:h + 1], scalar2=None,
                op0=Alu.mult)

    # ---- k projection (full, replicated): kT_sb [d, keys], true scale ----
    kT = acts.tile([P, NB, S], f8, tag="kT", name=f"kT{it}")
    for j in range(NB):
        pk = ps.tile([P, S], fp, tag="ps", name=f"pk{it}")
        for oo in range(ND):
            nc.tensor.matmul(pk[:],
                             lhsT=io["wk"][:, 2 * oo: 2 * oo + 2,
                                           P * j: P * (j + 1)],
                             rhs=hT[:, 2 * oo: 2 * oo + 2, :],
                             start=(oo == 0), stop=(oo == ND - 1),
                             perf_mode=DR)
        nc.scalar.activation(kT[:, j, :], pk[:], Act.Copy, scale=1.0 / WSCL)

    # ---- v (+ content bias) projection (full, replicated) ----
    v_sb = acts.tile([P, NKT, D], f8, tag="v", name=f"v{it}")
    bias_sb = acts.tile([P, NKT, H], fp, tag="bias", name=f"bias{it}")
    pcb = ps.tile([P, NKT, H], fp, tag="ps", name=f"pcb{it}")
    for kt in range(NKT):
        pv = [ps.tile([P, S], fp, tag="ps", name=f"pv{it}_{kt}_{eh}")
              for eh in range(2)]
        for oo in range(ND):
            lhs = hT[:, 2 * oo: 2 * oo + 2, P * kt: P * (kt + 1)]
            for eh in range(2):
                nc.tensor.matmul(pv[eh][:], lhsT=lhs,
                                 rhs=io["wvcb"][:, 2 * oo: 2 * oo + 2,
                                                S * eh: S * (eh + 1)],
                                 start=(oo == 0), stop=(oo == ND - 1),
                                 perf_mode=DR)
            nc.tensor.matmul(pcb[:, kt, :], lhsT=lhs,
                             rhs=io["wvcb"][:, 2 * oo: 2 * oo + 2, D:WCB],
                             start=(oo == 0), stop=(oo == ND - 1),
                             perf_mode=DR, skip_group_check=True)
        for eh in range(2):
            nc.scalar.activation(v_sb[:, kt, S * eh: S * (eh + 1)], pv[eh][:],
                                 Act.Copy, scale=1.0 / WSCL)
        # exp bias per key s and head: scale*cb[s, h] + ln(counts[s])
        nc.vector.scalar_tensor_tensor(
            out=bias_sb[:, kt, :], in0=pcb[:, kt, :], scalar=SCALE / WSCL,
            in1=io["lncnt_sb"][:, kt:kt + 1].to_broadcast([P, H]),
            op0=Alu.mult, op1=Alu.add)

    # ---- scores -> exp -> ctx/z, processed in two key-halves ----
    # scores^T layout [keys, qpack]: per pair-pair pp, psum [P, pa, ktl, 128];
    # the exp bias (scale*cb + ln counts) is per key = per partition.
    w_t = [[None] * 2 for _ in range(NPAIR)]   # [pair][kh]
    sc_t = [[None] * 2 for _ in range(NPP)]    # [pp][kh]
    cz = [None] * NPP                          # ctx+z accum, [pp]

    for kh in range(2):
        for pp in range(NPP):
            sc_t[pp][kh] = ps.tile([P, 2, 2, P], fp, tag="ps",
                                   name=f"sc{it}_{kh}_{pp}")
        # start=True clears the whole psum bank's has_written state, so only
        # the first matmul touching each tile may set it; later regions'
        # first writes land via has_written=0 overwrite semantics.
        for oo in range(ND):
            for ktl in range(2):
                kt = 2 * kh + ktl
                for pair in range(NPAIR):
                    nc.tensor.matmul(
                        sc_t[pair // 2][kh][:, pair % 2, ktl, :],
                        lhsT=kT[:, 2 * oo: 2 * oo + 2, P * kt: P * (kt + 1)],
                        rhs=mq[:, 2 * oo: 2 * oo + 2, pair],
                        start=(oo == 0 and ktl == 0 and pair % 2 == 0),
                        stop=(oo == ND - 1 and ktl == 1 and pair % 2 == 1),
                        perf_mode=DR, skip_group_check=True)
    # exp (ACT): per (pair, kh, ktl, head-half); bias differs per head.
    # psum holds 8*sc_true (mixing x8), so the activation scale is SCALE/8.
    for kh in range(2):
        for pair in range(NPAIR):
            wt = wp.tile([P, 2, P], f8, tag="w", name=f"w{it}_{kh}_{pair}")
            w_t[pair][kh] = wt
            for ktl in range(2):
                kt = 2 * kh + ktl
                for hh in range(2):
                    nc.scalar.activation(
                        wt[:, ktl, DH * hh: DH * (hh + 1)],
                        sc_t[pair // 2][kh][:, pair % 2, ktl,
                                            DH * hh: DH * (hh + 1)],
                        Act.Exp,
                        bias=bias_sb[:, kt, 2 * pair + hh: 2 * pair + hh + 1],
                        scale=SCALE / MSCL)
        # z and ctx accumulate across both key-halves
        if kh == 0:
            for pp in range(NPP):
                cz[pp] = ps.tile([P, 2, 2, P], fp, tag="ps",
                                 name=f"cz{it}_{pp}")
        for ktl in range(2):
            kt = 2 * kh + ktl
            for pair in range(NPAIR):   # z first: shared 1/32-valued lhsT
                nc.tensor.matmul(cz[pair // 2][:, pair % 2, 1, :],
                                 lhsT=io["ones_sb"][:],
                                 rhs=w_t[pair][kh][:, ktl, :],
                                 start=(kh == 0 and ktl == 0
                                        and pair % 2 == 0),
                                 stop=False,
                                 skip_group_check=True)
        for ktl in range(2):
            kt = 2 * kh + ktl
            for pair in range(NPAIR):
                nc.tensor.matmul(cz[pair // 2][:, pair % 2, 0, :],
                                 lhsT=v_sb[:, kt, P * pair: P * (pair + 1)],
                                 rhs=w_t[pair][kh][:, ktl, :],
                                 start=False,
                                 stop=(kh == 1 and ktl == 1
                                       and pair % 2 == 1),
                                 skip_group_check=True)

    # ---- normalize: ctxT[vcol, q] = 32*ctx[vcol, q]/z[q] ----
    # z psum holds z/32, so its reciprocal is 32/z directly. bv enters the
    # output dense linearly, so bv@Wd is folded into the host-side residual.
    ctxT = acts.tile([P, NB, Q], f8, tag="ctxT", name=f"ctxT{it}")
    for pair in range(NPAIR):
        pp, pa = pair // 2, pair % 2
        rzb = acts.tile([P, P], fp, tag="rzb", name=f"rzb{it}_{pair}")
        nc.vector.reciprocal(rzb[:], cz[pp][:, pa, 1, :])
        for hh in range(2):
            r0 = DH * hh
            nc.vector.tensor_tensor(ctxT[r0:r0 + DH, pair, :],
                                    cz[pp][r0:r0 + DH, pa, 0, r0:r0 + DH],
                                    rzb[r0:r0 + DH, r0:r0 + DH], Alu.mult)

    # ---- output projection (psum = 32*ctx_n @ 128*Wd = 4096*out) ----
    po = [ps.tile([Q, S], fp, tag="ps", name=f"po{it}_{eh}")
          for eh in range(2)]
    for oo in range(ND):
        for eh in range(2):
            nc.tensor.matmul(po[eh][:],
                             lhsT=ctxT[:, 2 * oo: 2 * oo + 2, :],
                             rhs=io["wd"][:, 2 * oo: 2 * oo + 2,
                                          S * eh: S * (eh + 1)],
                             start=(oo == 0), stop=(oo == ND - 1),
                             perf_mode=DR)

    # ---- epilogue: residual (hid+bd host-folded), LayerNorm, row-block out.
    # ln_gamma/ln_beta are ones/zeros in this problem; _run re-applies them
    # on the host if they ever deviate.
    r_sb = acts.tile([Q, D], fp, tag="r", name=f"r{it}")
    for eh in range(2):
        nc.vector.scalar_tensor_tensor(
            out=r_sb[:, S * eh: S * (eh + 1)], in0=po[eh][:],
            scalar=1.0 / (WSCL * ZSCL),
            in1=io["hidq_sb"][:, S * eh: S * (eh + 1)],
            op0=Alu.mult, op1=Alu.add)
    stats = acts.tile([Q, 2, 6], fp, tag="stats", name=f"stats{it}")
    nc.vector.bn_stats(stats[:, 0, :], r_sb[:, 0:S])
    nc.vector.bn_stats(stats[:, 1, :], r_sb[:, S:D])
    mv = acts.tile([Q, 2], fp, tag="mv", name=f"mv{it}")
    nc.vector.bn_aggr(mv[:], stats[:])
    std = acts.tile([Q, 1], fp, tag="std", name=f"std{it}")
    nc.scalar.activation(std[:], mv[:, 1:2], Act.Sqrt,
                         bias=io["eps_t"][0:Q, :], scale=1.0)
    nc.vector.reciprocal(std[:], std[:])
    nc.vector.tensor_scalar(out=r_sb[:], in0=r_sb[:],
                            scalar1=mv[:, 0:1], scalar2=std[:],
                            op0=Alu.subtract, op1=Alu.mult)
    nc.sync.dma_start(io["out"][:], r_sb[:])


def _build(iters=1):
    import concourse.bass as bass
    import concourse.mybir as mybir
    import concourse.tile as tile
    from concourse import bacc

    fp = mybir.dt.float32
    f8 = mybir.dt.float8e4

    nc = bacc.Bacc("TRN2", target_bir_lowering=False, debug=False,
                   num_devices=N_CORES)

    hTd = nc.dram_tensor("hT", [D, S], f8, kind="ExternalInput").ap()
    hTqd = nc.dram_tensor("hTq", [D, Q], f8, kind="ExternalInput").ap()
    hidqd = nc.dram_tensor("hidq", [Q, D], fp, kind="ExternalInput").ap()
    wkd = nc.dram_tensor("wk", [D, D], f8, kind="ExternalInput").ap()
    wqd = nc.dram_tensor("wq", [D, D], f8, kind="ExternalInput").ap()
    wvcbd = nc.dram_tensor("wvcb", [D, WCB], f8, kind="ExternalInput").ap()
    wdd = nc.dram_tensor("wd", [D, D], f8, kind="ExternalInput").ap()
    mixtd = nc.dram_tensor("mixt", [D, H], fp, kind="ExternalInput").ap()
    lncntd = nc.dram_tensor("lncnt", [S], fp, kind="ExternalInput").ap()
    identd = nc.dram_tensor("ident", [Q, Q], mybir.dt.bfloat16, kind="ExternalInput").ap()
    outd = nc.dram_tensor("out", [Q, D], fp, kind="ExternalOutput").ap()

    def bcast_dram(vec_ap, parts):
        # DMA-replicate a [n] DRAM vector across `parts` partitions.
        return bass.AP(tensor=vec_ap.tensor, offset=vec_ap.offset,
                       ap=[[0, parts]] + [list(d) for d in vec_ap.ap])

    with tile.TileContext(nc) as tc:
        with (
            tc.tile_pool(name="singles", bufs=1) as singles,
            tc.tile_pool(name="acts", bufs=2) as acts,
            tc.tile_pool(name="mqp", bufs=2) as mqp,
            tc.tile_pool(name="wp", bufs=16) as wp,
            tc.tile_pool(name="ps", bufs=8, space="PSUM") as ps,
        ):
            pools = {"singles": singles, "acts": acts, "mqp": mqp,
                     "wp": wp, "ps": ps}
            # ---- constant / input loads (once) ----
            hT = singles.tile([P, NB, S], f8)
            hTr = hTd.rearrange("(o p) u -> p o u", p=P)
            for _o in range(NB):
                nc.sync.dma_start(hT[:, _o, :], hTr[:, _o, :])
            hTq = singles.tile([P, NB, Q], f8)
            nc.sync.dma_start(hTq[:], hTqd.rearrange("(o p) q -> p o q", p=P))
            wk_sb = singles.tile([P, NB, D], f8)
            wq_sb = singles.tile([P, NB, D], f8)
            wd_sb = singles.tile([P, NB, D], f8)
            for wdram, dest in ((wkd, wk_sb), (wqd, wq_sb), (wdd, wd_sb)):
                wr = wdram.rearrange("(o p) m -> p o m", p=P)
                for _o in range(NB):
                    nc.sync.dma_start(dest[:, _o, :], wr[:, _o, :])
            wvcb_sb = singles.tile([P, NB, WCB], f8)
            wvr = wvcbd.rearrange("(o p) m -> p o m", p=P)
            for _o in range(NB):
                nc.sync.dma_start(wvcb_sb[:, _o, :], wvr[:, _o, :])
            mixt_sb = singles.tile([P, NB, H], fp)
            nc.sync.dma_start(mixt_sb[:],
                              mixtd.rearrange("(o p) h -> p o h", p=P))
            lncnt_sb = singles.tile([P, NKT], fp)
            nc.sync.dma_start(lncnt_sb[:],
                              lncntd.rearrange("(a p) -> p a", p=P))
            hidq_sb = singles.tile([Q, D], fp)
            nc.sync.dma_start(hidq_sb[:], hidqd)
            id64 = singles.tile([Q, Q], mybir.dt.bfloat16)
            nc.sync.dma_start(id64[:], identd)
            ones_f32 = singles.tile([P, P], fp)
            nc.vector.memset(ones_f32[:], 1.0 / ZSCL)
            ones_sb = singles.tile([P, P], f8)
            with nc.allow_low_precision(reason="exact 1/32 constants to fp8"):
                nc.vector.tensor_copy(ones_sb[:], ones_f32[:])
            eps_t = singles.tile([P, 1], fp)
            nc.vector.memset(eps_t[:], LN_EPS)

            io = {"hT": hT, "hTq": hTq, "wk": wk_sb, "wq": wq_sb,
                  "wvcb": wvcb_sb, "wd": wd_sb, "mixt_sb": mixt_sb,
                  "lncnt_sb": lncnt_sb, "hidq_sb": hidq_sb,
                  "id64": id64, "ones_sb": ones_sb, "eps_t": eps_t,
                  "out": outd}

            with nc.allow_low_precision(
                    reason="attention path tolerates fp8; residual+LN fp32"):
                for it in range(iters):
                    _emit(nc, tc, pools, io, it)

    nc.compile()
    return nc


def _get_nc(iters=1):
    key = ("nc", iters)
    if key not in _CACHE:
        _CACHE[key] = _build(iters)
    return _CACHE[key]


def _prepare_in_maps(hidden_states, fpos, tpos, Wq, Wk, Wcb, Wv, bv, mixing,
                     Wd, bd, ln_gamma, ln_beta):
    import ml_dtypes
    f8 = ml_dtypes.float8_e4m3
    hs = np.ascontiguousarray(np.asarray(hidden_states, dtype=np.float32))
    tidx = np.asarray(tpos).astype(np.int64) % S
    counts = np.bincount(tidx, minlength=S).astype(np.float64)
    lncnt = np.where(counts > 0, np.log(np.maximum(counts, 1e-30)),
                     NEG_BIG).astype(np.float32)
    wvcb = np.concatenate([np.asarray(Wv, np.float32),
                           np.asarray(Wcb, np.float32)], axis=1)
    common = {
        "hT": np.ascontiguousarray(hs.T).astype(f8),
        "wk": (WSCL * np.asarray(Wk, np.float32)).astype(f8),
        "wq": (WSCL * np.asarray(Wq, np.float32)).astype(f8),
        "wvcb": (WSCL * wvcb).astype(f8),
        "wd": (WSCL * np.asarray(Wd, np.float32)).astype(f8),
        "mixt": np.ascontiguousarray(
            MSCL * np.asarray(mixing, np.float32).T),
        "lncnt": lncnt,
        "ident": np.eye(Q, dtype=np.float32).astype(ml_dtypes.bfloat16),
    }
    resid_bias = (np.asarray(bd, np.float32)
                  + np.asarray(bv, np.float32)
                  @ np.asarray(Wd, np.float32))
    in_maps = []
    for c in range(N_CORES):
        m = dict(common)
        rows = hs[Q * c: Q * (c + 1)]
        m["hTq"] = np.ascontiguousarray(rows.T).astype(f8)
        m["hidq"] = np.ascontiguousarray(rows + resid_bias[None, :])
        in_maps.append(m)
    return in_maps


def _run(inputs, trace=False, iters=1):
    from concourse import bass_utils
    nc = _get_nc(iters)
    in_maps = _prepare_in_maps(**inputs)
    res = bass_utils.run_bass_kernel_spmd(
        nc, in_maps, core_ids=list(range(N_CORES)), trace=trace)
    normedfull = np.concatenate(
        [np.asarray(res.results[c]["out"]) for c in range(N_CORES)], axis=0)
    gam = np.asarray(inputs["ln_gamma"], np.float32)
    bet = np.asarray(inputs["ln_beta"], np.float32)
    if not (np.all(gam == 1.0) and np.all(bet == 0.0)):
        normedfull = normedfull * gam[None, :] + bet[None, :]
    fidx = np.asarray(inputs["fpos"]).astype(np.int64) % S
    return np.ascontiguousarray(normedfull[fidx]), res


def kernel(**inputs) -> np.ndarray:
    out, _ = _run(inputs, trace=False)
    return out


# revision 14
# speedup vs baseline: 49.3528x; 3.8374x over previous
"""Trainium2 Bass kernel for CollaborativeAttention (row-sharded, fp8).

Math: with S=512 unique positions and F=T=2048 gathered via fpos/tpos (mod 512),
the whole block collapses to the unique-position problem:
    qf = hs @ Wq ; kf = hs @ Wk ; vf = hs @ Wv + bv ; cbf = hs @ Wcb       [512, *]
    per head h:  w[u, s] = counts[s] * exp(scale*(qf[u]*mix[h]) . kf[s]
                                           + scale*cbf[s, h])
    ctx[u, h*64:(h+1)*64] = (w @ vf[:, h*64:(h+1)*64]) / w.sum(axis=1)
    outfull = ctx @ Wd + bd ; resfull = hs + outfull ; LN  -> normedfull   [512, 1024]
    output  = normedfull[fpos % 512]                                       [2048, 1024]
counts[s] = multiplicity of s in (tpos % 512); softmax over the 2048 keys is
exactly the count-weighted softmax over the 512 unique keys.

Distribution: collectives on this stack are far too slow (~ms), so the kernel
uses a zero-collective row shard: core c owns query rows 64c..64c+63. The k/v
projections (needed in full by every core) are replicated; everything else
(q, scores, softmax, context, output dense, LayerNorm) is 1/8 per core. The
host concatenates the 8 disjoint row blocks and applies the fpos gather.

Precision: the residual path keeps the attention output at ~2% of the signal,
so the whole attention path runs in fp8 e4m3 with DoubleRow matmuls (fp32
PSUM accumulation); the residual + LayerNorm stay fp32. Power-of-2 scale
factors keep fp8 operands in the normal range: weights are pre-scaled x128 on
the host (compensated at PSUM eviction), mixing x8 (compensated in the exp
scale), the softmax reciprocal is computed as 32/z (all-ones lhsT holds 1/32)
so the stored context is x32 (compensated by a /4096 at the output-dense
eviction). bd and bv@Wd enter the output linearly and are folded into the
host-prepared residual rows; ln_gamma/ln_beta are ones/zeros here (the host
re-applies them if they ever deviate). All weights are SBUF-resident.
"""

import math
import numpy as np

P = 128
S = 512
D = 1024
H = 16
DH = 64
Q = 64               # query rows per core
NB = D // P          # 8 contraction chunks
ND = NB // 2         # 4 DoubleRow double-chunks
NKT = S // P         # 4 key tiles
NPAIR = H // 2       # 8 head pairs
NPP = NPAIR // 2     # 4 pair-pairs (2 pairs share a psum slot)
WCB = D + H          # packed [Wv | Wcb] columns
N_CORES = 8
SCALE = 1.0 / math.sqrt(D / H)  # 0.125
LN_EPS = 1e-5
NEG_BIG = -30000.0

WSCL = 128.0         # host weight scale (power of 2, keeps fp8 normal-range)
MSCL = 8.0           # host mixing scale
ZSCL = 32.0          # context scale via ones=1/ZSCL z-reduction

_CACHE = {}


def _emit(nc, tc, pools, io, it):
    """Emit one full compute iteration (everything after the constant loads)."""
    import concourse.mybir as mybir

    fp = mybir.dt.float32
    f8 = mybir.dt.float8e4
    bf = mybir.dt.bfloat16
    Alu = mybir.AluOpType
    Act = mybir.ActivationFunctionType
    DR = mybir.MatmulPerfMode.DoubleRow

    acts, wp, ps = (pools[k] for k in ("acts", "wp", "ps"))
    hT = io["hT"]

    # ---- q projection (local 64 rows): q_sb [Q, D], true scale ----
    q_sb = acts.tile([Q, D], bf, tag="q", name=f"q{it}")
    for eh in range(2):
        pq = ps.tile([Q, S], fp, tag="ps", name=f"pq{it}")
        for oo in range(ND):
            nc.tensor.matmul(pq[:], lhsT=io["hTq"][:, 2 * oo: 2 * oo + 2, :],
                             rhs=io["wq"][:, 2 * oo: 2 * oo + 2,
                                          S * eh: S * (eh + 1)],
                             start=(oo == 0), stop=(oo == ND - 1),
                             perf_mode=DR)
        nc.scalar.activation(q_sb[:, S * eh: S * (eh + 1)], pq[:],
                             Act.Copy, scale=1.0 / WSCL)

    # qT via PE transpose: qTp[:, o, :] = q_sb[:, 128o:128(o+1)]^T
    qTp = ps.tile([P, NB, Q], bf, tag="ps", name=f"qTp{it}")
    for o in range(NB):
        nc.tensor.transpose(qTp[:, o, :], q_sb[:, P * o: P * (o + 1)],
                            io["id64"][:])
    # one bulk psum->SBUF copy so the 128 mq ops read SBUF (58-cycle DVE
    # overhead) instead of PSUM (120-cycle)
    qT_sb = acts.tile([P, NB, Q], bf, tag="qT", name=f"qT{it}")
    nc.scalar.copy(qT_sb[:], qTp.rearrange("p a b -> p (a b)"))
    # mq[d, pair, hh, q] = qT[d, q] * (8*mixing[h, d])  (DVE per-part scalar)
    mq = pools["mqp"].tile([P, NB, NPAIR, 2, Q], f8, tag="mq", name=f"mq{it}")
    for o in range(NB):
        for h in range(H):
            nc.vector.tensor_scalar(
                out=mq[:, o, h // 2, h % 2, :], in0=qT_sb[:, o, :],
                scalar1=io["mixt_sb"][:, o, h:h + 1], scalar2=None,
                op0=Alu.mult)

    # ---- k projection (full, replicated): kT_sb [d, keys], true scale ----
    kT = acts.tile([P, NB, S], f8, tag="kT", name=f"kT{it}")
    for j in range(NB):
        pk = ps.tile([P, S], fp, tag="ps", name=f"pk{it}")
        for oo in range(ND):
            nc.tensor.matmul(pk[:],
                             lhsT=io["wk"][:, 2 * oo: 2 * oo + 2,
                                           P * j: P * (j + 1)],
                             rhs=hT[:, 2 * oo: 2 * oo + 2, :],
                             start=(oo == 0), stop=(oo == ND - 1),
                             perf_mode=DR)
        nc.scalar.activation(kT[:, j, :], pk[:], Act.Copy, scale=1.0 / WSCL)

    # ---- v (+ content bias) projection (full, replicated) ----
    v_sb = acts.tile([P, NKT, D], f8, tag="v", name=f"v{it}")
    bias_sb = acts.tile([P, NKT, H], fp, tag="bias", name=f"bias{it}")
    pcb = ps.tile([P, NKT, H], fp, tag="ps", name=f"pcb{it}")
    for kt in range(NKT):
        pv = [ps.tile([P, S], fp, tag="ps", name=f"pv{it}_{kt}_{eh}")
              for eh in range(2)]
        for oo in range(ND):
            lhs = hT[:, 2 * oo: 2 * oo + 2, P * kt: P * (kt + 1)]
            for eh in range(2):
                nc.tensor.matmul(pv[eh][:], lhsT=lhs,
                                 rhs=io["wvcb"][:, 2 * oo: 2 * oo + 2,
                                                S * eh: S * (eh + 1)],
                                 start=(oo == 0), stop=(oo == ND - 1),
                                 perf_mode=DR)
            nc.tensor.matmul(pcb[:, kt, :], lhsT=lhs,
                             rhs=io["wvcb"][:, 2 * oo: 2 * oo + 2, D:WCB],
                             start=(oo == 0), stop=(oo == ND - 1),
                             perf_mode=DR, skip_group_check=True)
        for eh in range(2):
            nc.scalar.activation(v_sb[:, kt, S * eh: S * (eh + 1)], pv[eh][:],
                                 Act.Copy, scale=1.0 / WSCL)
        # exp bias per key s and head: scale*cb[s, h] + ln(counts[s])
        nc.vector.scalar_tensor_tensor(
            out=bias_sb[:, kt, :], in0=pcb[:, kt, :], scalar=SCALE / WSCL,
            in1=io["lncnt_sb"][:, kt:kt + 1].to_broadcast([P, H]),
            op0=Alu.mult, op1=Alu.add)

    # ---- scores -> exp -> ctx/z, processed in two key-halves ----
    # scores^T layout [keys, qpack]: per pair-pair pp, psum [P, pa, ktl, 128];
    # the exp bias (scale*cb + ln counts) is per key = per partition.
    w_t = [[None] * 2 for _ in range(NPAIR)]   # [pair][kh]
    sc_t = [[None] * 2 for _ in range(NPP)]    # [pp][kh]
    cz = [None] * NPP                          # ctx+z accum, [pp]

    for kh in range(2):
        for pp in range(NPP):
            sc_t[pp][kh] = ps.tile([P, 2, 2, P], fp, tag="ps",
                                   name=f"sc{it}_{kh}_{pp}")
        # start=True clears the whole psum bank's has_written state, so only
        # the first matmul touching each tile may set it; later regions'
        # first writes land via has_written=0 overwrite semantics.
        for oo in range(ND):
            for ktl in range(2):
                kt = 2 * kh + ktl
                for pair in range(NPAIR):
                    nc.tensor.matmul(
                        sc_t[pair // 2][kh][:, pair % 2, ktl, :],
                        lhsT=kT[:, 2 * oo: 2 * oo + 2, P * kt: P * (kt + 1)],
                        rhs=mq[:, 2 * oo: 2 * oo + 2, pair],
                        start=(oo == 0 and ktl == 0 and pair % 2 == 0),
                        stop=(oo == ND - 1 and ktl == 1 and pair % 2 == 1),
                        perf_mode=DR, skip_group_check=True)
    # exp (ACT): per (pair, kh, ktl, head-half); bias differs per head.
    # psum holds 8*sc_true (mixing x8), so the activation scale is SCALE/8.
    for kh in range(2):
        for pair in range(NPAIR):
            wt = wp.tile([P, 2, P], f8, tag="w", name=f"w{it}_{kh}_{pair}")
            w_t[pair][kh] = wt
            for ktl in range(2):
                kt = 2 * kh + ktl
                for hh in range(2):
                    nc.scalar.activation(
                        wt[:, ktl, DH * hh: DH * (hh + 1)],
                        sc_t[pair // 2][kh][:, pair % 2, ktl,
                                            DH * hh: DH * (hh + 1)],
                        Act.Exp,
                        bias=bias_sb[:, kt, 2 * pair + hh: 2 * pair + hh + 1],
                        scale=SCALE / MSCL)
        # z and ctx accumulate across both key-halves
        if kh == 0:
            for pp in range(NPP):
                cz[pp] = ps.tile([P, 2, 2, P], fp, tag="ps",
                                 name=f"cz{it}_{pp}")
        for ktl in range(2):
            kt = 2 * kh + ktl
            for pair in range(NPAIR):   # z first: shared 1/32-valued lhsT
                nc.tensor.matmul(cz[pair // 2][:, pair % 2, 1, :],
                                 lhsT=io["ones_sb"][:],
                                 rhs=w_t[pair][kh][:, ktl, :],
                                 start=(kh == 0 and ktl == 0
                                        and pair % 2 == 0),
                                 stop=False,
                                 skip_group_check=True)
        for ktl in range(2):
            kt = 2 * kh + ktl
            for pair in range(NPAIR):
                nc.tensor.matmul(cz[pair // 2][:, pair % 2, 0, :],
                                 lhsT=v_sb[:, kt, P * pair: P * (pair + 1)],
                                 rhs=w_t[pair][kh][:, ktl, :],
                                 start=False,
                                 stop=(kh == 1 and ktl == 1
                                       and pair % 2 == 1),
                                 skip_group_check=True)

    # ---- normalize: ctxT[vcol, q] = 32*ctx[vcol, q]/z[q] ----
    # z psum holds z/32, so its reciprocal is 32/z directly. bv enters the
    # output dense linearly, so bv@Wd is folded into the host-side residual.
    ctxT = acts.tile([P, NB, Q], f8, tag="ctxT", name=f"ctxT{it}")
    for pair in range(NPAIR):
        pp, pa = pair // 2, pair % 2
        rzb = acts.tile([P, P], fp, tag="rzb", name=f"rzb{it}_{pair}")
        nc.vector.reciprocal(rzb[:], cz[pp][:, pa, 1, :])
        for hh in range(2):
            r0 = DH * hh
            nc.vector.tensor_tensor(ctxT[r0:r0 + DH, pair, :],
                                    cz[pp][r0:r0 + DH, pa, 0, r0:r0 + DH],
                                    rzb[r0:r0 + DH, r0:r0 + DH], Alu.mult)

    # ---- output projection (psum = 32*ctx_n @ 128*Wd = 4096*out) ----
    po = [ps.tile([Q, S], fp, tag="ps", name=f"po{it}_{eh}")
          for eh in range(2)]
    for oo in range(ND):
        for eh in range(2):
            nc.tensor.matmul(po[eh][:],
                             lhsT=ctxT[:, 2 * oo: 2 * oo + 2, :],
                             rhs=io["wd"][:, 2 * oo: 2 * oo + 2,
                                          S * eh: S * (eh + 1)],
                             start=(oo == 0), stop=(oo == ND - 1),
                             perf_mode=DR)

    # ---- epilogue: residual (hid+bd host-folded), LayerNorm, row-block out.
    # ln_gamma/ln_beta are ones/zeros in this problem; _run re-applies them
    # on the host if they ever deviate.
    r_sb = acts.tile([Q, D], fp, tag="r", name=f"r{it}")
    for eh in range(2):
        nc.vector.scalar_tensor_tensor(
            out=r_sb[:, S * eh: S * (eh + 1)], in0=po[eh][:],
            scalar=1.0 / (WSCL * ZSCL),
            in1=io["hidq_sb"][:, S * eh: S * (eh + 1)],
            op0=Alu.mult, op1=Alu.add)
    stats = acts.tile([Q, 2, 6], fp, tag="stats", name=f"stats{it}")
    nc.vector.bn_stats(stats[:, 0, :], r_sb[:, 0:S])
    nc.vector.bn_stats(stats[:, 1, :], r_sb[:, S:D])
    mv = acts.tile([Q, 2], fp, tag="mv", name=f"mv{it}")
    nc.vector.bn_aggr(mv[:], stats[:])
    std = acts.tile([Q, 1], fp, tag="std", name=f"std{it}")
    nc.scalar.activation(std[:], mv[:, 1:2], Act.Sqrt,
                         bias=io["eps_t"][0:Q, :], scale=1.0)
    nc.vector.reciprocal(std[:], std[:])
    nc.vector.tensor_scalar(out=r_sb[:], in0=r_sb[:],
                            scalar1=mv[:, 0:1], scalar2=std[:],
                            op0=Alu.subtract, op1=Alu.mult)
    nc.sync.dma_start(io["out"][:], r_sb[:])


def _build(iters=1):
    import concourse.mybir as mybir
    import concourse.tile as tile
    from concourse import bacc

    fp = mybir.dt.float32
    f8 = mybir.dt.float8e4

    nc = bacc.Bacc("TRN2", target_bir_lowering=False, debug=False,
                   num_devices=N_CORES)

    hTd = nc.dram_tensor("hT", [D, S], f8, kind="ExternalInput").ap()
    hTqd = nc.dram_tensor("hTq", [D, Q], f8, kind="ExternalInput").ap()
    hidqd = nc.dram_tensor("hidq", [Q, D], fp, kind="ExternalInput").ap()
    wkd = nc.dram_tensor("wk", [D, D], f8, kind="ExternalInput").ap()
    wqd = nc.dram_tensor("wq", [D, D], f8, kind="ExternalInput").ap()
    wvcbd = nc.dram_tensor("wvcb", [D, WCB], f8, kind="ExternalInput").ap()
    wdd = nc.dram_tensor("wd", [D, D], f8, kind="ExternalInput").ap()
    mixtd = nc.dram_tensor("mixt", [D, H], fp, kind="ExternalInput").ap()
    lncntd = nc.dram_tensor("lncnt", [S], fp, kind="ExternalInput").ap()
    identd = nc.dram_tensor("ident", [Q, Q], mybir.dt.bfloat16, kind="ExternalInput").ap()
    outd = nc.dram_tensor("out", [Q, D], fp, kind="ExternalOutput").ap()

    with tile.TileContext(nc) as tc:
        with (
            tc.tile_pool(name="singles", bufs=1) as singles,
            tc.tile_pool(name="acts", bufs=2) as acts,
            tc.tile_pool(name="mqp", bufs=2) as mqp,
            tc.tile_pool(name="wp", bufs=16) as wp,
            tc.tile_pool(name="ps", bufs=8, space="PSUM") as ps,
        ):
            pools = {"singles": singles, "acts": acts, "mqp": mqp,
                     "wp": wp, "ps": ps}
            # ---- constant / input loads (once) ----
            hT = singles.tile([P, NB, S], f8)
            hTr = hTd.rearrange("(o p) u -> p o u", p=P)
            for _o in range(NB):
                nc.sync.dma_start(hT[:, _o, :], hTr[:, _o, :])
            hTq = singles.tile([P, NB, Q], f8)
            nc.sync.dma_start(hTq[:], hTqd.rearrange("(o p) q -> p o q", p=P))
            wk_sb = singles.tile([P, NB, D], f8)
            wq_sb = singles.tile([P, NB, D], f8)
            wd_sb = singles.tile([P, NB, D], f8)
            for wdram, dest in ((wkd, wk_sb), (wqd, wq_sb), (wdd, wd_sb)):
                wr = wdram.rearrange("(o p) m -> p o m", p=P)
                for _o in range(NB):
                    nc.sync.dma_start(dest[:, _o, :], wr[:, _o, :])
            wvcb_sb = singles.tile([P, NB, WCB], f8)
            wvr = wvcbd.rearrange("(o p) m -> p o m", p=P)
            for _o in range(NB):
                nc.sync.dma_start(wvcb_sb[:, _o, :], wvr[:, _o, :])
            mixt_sb = singles.tile([P, NB, H], fp)
            nc.sync.dma_start(mixt_sb[:],
                              mixtd.rearrange("(o p) h -> p o h", p=P))
            lncnt_sb = singles.tile([P, NKT], fp)
            nc.sync.dma_start(lncnt_sb[:],
                              lncntd.rearrange("(a p) -> p a", p=P))
            hidq_sb = singles.tile([Q, D], fp)
            nc.sync.dma_start(hidq_sb[:], hidqd)
            id64 = singles.tile([Q, Q], mybir.dt.bfloat16)
            nc.sync.dma_start(id64[:], identd)
            ones_f32 = singles.tile([P, P], fp)
            nc.vector.memset(ones_f32[:], 1.0 / ZSCL)
            ones_sb = singles.tile([P, P], f8)
            with nc.allow_low_precision(reason="exact 1/32 constants to fp8"):
                nc.vector.tensor_copy(ones_sb[:], ones_f32[:])
            eps_t = singles.tile([P, 1], fp)
            nc.vector.memset(eps_t[:], LN_EPS)

            io = {"hT": hT, "hTq": hTq, "wk": wk_sb, "wq": wq_sb,
                  "wvcb": wvcb_sb, "wd": wd_sb, "mixt_sb": mixt_sb,
                  "lncnt_sb": lncnt_sb, "hidq_sb": hidq_sb,
                  "id64": id64, "ones_sb": ones_sb, "eps_t": eps_t,
                  "out": outd}

            with nc.allow_low_precision(
                    reason="attention path tolerates fp8; residual+LN fp32"):
                for it in range(iters):
                    _emit(nc, tc, pools, io, it)

    nc.compile()
    return nc


def _get_nc(iters=1):
    key = ("nc", iters)
    if key not in _CACHE:
        _CACHE[key] = _build(iters)
    return _CACHE[key]


def _prepare_in_maps(hidden_states, fpos, tpos, Wq, Wk, Wcb, Wv, bv, mixing,
                     Wd, bd, ln_gamma, ln_beta):
    import ml_dtypes
    f8 = ml_dtypes.float8_e4m3
    hs = np.ascontiguousarray(np.asarray(hidden_states, dtype=np.float32))
    tidx = np.asarray(tpos).astype(np.int64) % S
    counts = np.bincount(tidx, minlength=S).astype(np.float64)
    lncnt = np.where(counts > 0, np.log(np.maximum(counts, 1e-30)),
                     NEG_BIG).astype(np.float32)
    wvcb = np.concatenate([np.asarray(Wv, np.float32),
                           np.asarray(Wcb, np.float32)], axis=1)
    common = {
        "hT": np.ascontiguousarray(hs.T).astype(f8),
        "wk": (WSCL * np.asarray(Wk, np.float32)).astype(f8),
        "wq": (WSCL * np.asarray(Wq, np.float32)).astype(f8),
        "wvcb": (WSCL * wvcb).astype(f8),
        "wd": (WSCL * np.asarray(Wd, np.float32)).astype(f8),
        "mixt": np.ascontiguousarray(
            MSCL * np.asarray(mixing, np.float32).T),
        "lncnt": lncnt,
        "ident": np.eye(Q, dtype=np.float32).astype(ml_dtypes.bfloat16),
    }
    resid_bias = (np.asarray(bd, np.float32)
                  + np.asarray(bv, np.float32)
                  @ np.asarray(Wd, np.float32))
    in_maps = []
    for c in range(N_CORES):
        m = dict(common)
        rows = hs[Q * c: Q * (c + 1)]
        m["hTq"] = np.ascontiguousarray(rows.T).astype(f8)
        m["hidq"] = np.ascontiguousarray(rows + resid_bias[None, :])
        in_maps.append(m)
    return in_maps


def _run(inputs, trace=False, iters=1):
    from concourse import bass_utils
    nc = _get_nc(iters)
    in_maps = _prepare_in_maps(**inputs)
    res = bass_utils.run_bass_kernel_spmd(
        nc, in_maps, core_ids=list(range(N_CORES)), trace=trace)
    normedfull = np.concatenate(
        [np.asarray(res.results[c]["out"]) for c in range(N_CORES)], axis=0)
    gam = np.asarray(inputs["ln_gamma"], np.float32)
    bet = np.asarray(inputs["ln_beta"], np.float32)
    if not (np.all(gam == 1.0) and np.all(bet == 0.0)):
        normedfull = normedfull * gam[None, :] + bet[None, :]
    fidx = np.asarray(inputs["fpos"]).astype(np.int64) % S
    return np.ascontiguousarray(normedfull[fidx]), res


def kernel(**inputs) -> np.ndarray:
    out, _ = _run(inputs, trace=False)
    return out
